# revision 7
# baseline (speedup 1.0000x reference)
"""Trainium2 Bass kernel for nn_DependencyParser.

Pipeline (per NeuronCore, SPMD on 8 cores):
  - on-device embedding gather (dma_gather; paired-row trick since the 50000
    vocab exceeds int16 index range)
  - 2-layer BiLSTM solved by Jacobi fixed-point iteration over the hidden
    sequence: each sweep is a batched [2048,512]x[512,512] matmul + gate
    activations; the cell state c is computed EXACTLY per sweep with the
    hardware linear-scan op (tensor_tensor_scan).  The iteration contracts
    at ~0.55x per sweep; K=12 sweeps/layer -> ~1e-3 relative score error.
  - head MLP, then pairwise scores tanh(mlp[i]+mlp[j+1]) @ out_w + out_b,
    sharded over heads: core c computes score rows [64c, 64c+64).
All sequence tensors are in [feature, time] layout (feature on partitions).
"""
import sys
sys.path.insert(0, '/opt/trn_rl_repo')
import numpy as np

import concourse.bass as bass
import concourse.mybir as mybir
import concourse.tile as tile
from concourse import bacc
from concourse.masks import make_identity
from concourse.bass_utils import run_bass_kernel_spmd

F32 = mybir.dt.float32
I16 = mybir.dt.int16
AF = mybir.ActivationFunctionType
OP = mybir.AluOpType

L = 512          # sequence length
NG = 2048        # gate width 4*H
V2 = 25000       # paired vocab rows
WD, PD = 256, 64
DIN = WD + PD
M = 512          # mlp width
NCORES = 8
HPC = L // NCORES  # heads per core
K0 = 12          # jacobi sweeps, layer 0
K1 = 12          # jacobi sweeps, layer 1

_CACHE = {}


def _emit_lstm_dir(nc, sp_, wp, psum, pre, bT, whh, n_sweeps):
    """Jacobi-iterate one LSTM direction; returns [128, 4, 512] hidden states.

    pre: [128, 16, 512] W_ih @ x^T (no bias), n-tile major.
    bT:  [128, 16] gate bias. whh: [128, 4, 2048] lhsT layout.
    The ping-pong h buffers store h shifted by one step (col t holds h_{t-1})
    so the recurrent matmul uses plain aligned APs; the final sweep writes
    unshifted output.
    """
    sig, tanh = AF.Sigmoid, AF.Tanh
    hA = sp_.tile([128, 4, L], F32, tag="hA")
    hB = sp_.tile([128, 4, L], F32, tag="hB")
    nc.gpsimd.memset(hA[:, :, 0:1], 0.0)
    nc.gpsimd.memset(hB[:, :, 0:1], 0.0)
    for s in range(n_sweeps):
        hprev, hnew = (hA, hB) if s % 2 == 0 else (hB, hA)
        last = s == n_sweeps - 1
        for ht in range(4):
            gates = []
            for gi, nt in enumerate((ht, 4 + ht, 8 + ht, 12 + ht)):  # i,f,g,o
                func = tanh if gi == 2 else sig
                gate = wp.tile([128, L], F32, tag=f"g{gi}")
                if s == 0:
                    nc.scalar.activation(gate[:], pre[:, nt, :], func,
                                         bias=bT[:, nt:nt + 1])
                else:
                    zp = psum.tile([128, L], F32, tag="ps")
                    for kt in range(4):
                        nc.tensor.matmul(zp[:], whh[:, kt, nt * 128:(nt + 1) * 128],
                                         hprev[:, kt, :], start=(kt == 0),
                                         stop=(kt == 3))
                    nc.vector.tensor_tensor(zp[:], zp[:], pre[:, nt, :], OP.add)
                    nc.scalar.activation(gate[:], zp[:], func,
                                         bias=bT[:, nt:nt + 1])
                gates.append(gate)
            gi_, gf_, gg_, go_ = gates
            u = wp.tile([128, L], F32, tag="u")
            nc.vector.tensor_tensor(u[:], gi_[:], gg_[:], OP.mult)
            c = wp.tile([128, L], F32, tag="c")
            nc.vector.tensor_tensor_scan(c[:], gf_[:], u[:], 0.0, OP.mult, OP.add)
            tc_ = wp.tile([128, L], F32, tag="tc")
            nc.scalar.activation(tc_[:], c[:], tanh)
            if last:
                nc.vector.tensor_tensor(hnew[:, ht, :], go_[:], tc_[:], OP.mult)
            else:
                nc.vector.tensor_tensor(hnew[:, ht, 1:L], go_[:, 0:L - 1],
                                        tc_[:, 0:L - 1], OP.mult)
    return hA if n_sweeps % 2 == 0 else hB


def _build_program():
    nc = bacc.Bacc("TRN2", target_bir_lowering=False, debug=False,
                   num_devices=NCORES)

    def dram_in(name, shape, dtype=F32):
        return nc.dram_tensor(name, shape, dtype, kind="ExternalInput")

    w2_d = dram_in("w2", [V2, 2 * WD])
    pemb_d = dram_in("pemb", [50, PD])
    idx_d = {}
    for sfx in ("", "r"):
        idx_d["w" + sfx] = dram_in(f"widx{sfx}", [128, 32], I16)
        idx_d["p" + sfx] = dram_in(f"pidx{sfx}", [128, 32], I16)
        idx_d["m" + sfx] = dram_in(f"wpar{sfx}", [128, 4, 1])
    wih0_d = {d: dram_in(f"wih0{d}", [128, 3, NG]) for d in "fb"}
    whh0_d = {d: dram_in(f"whh0{d}", [128, 4, NG]) for d in "fb"}
    b0_d = {d: dram_in(f"b0{d}", [128, 16]) for d in "fb"}
    wih1_d = {d: dram_in(f"wih1{d}", [128, 8, NG]) for d in "fb"}
    whh1_d = {d: dram_in(f"whh1{d}", [128, 4, NG]) for d in "fb"}
    b1_d = {d: dram_in(f"b1{d}", [128, 16]) for d in "fb"}
    mlpw_d = dram_in("mlpw", [128, 8, M])
    mlpb2_d = dram_in("mlpb2", [128, 4])
    outw_d = dram_in("outw", [128, 4])
    outb_d = dram_in("outb", [1, 1])
    sel_d = dram_in("sel", [128, 4, HPC])
    out_d = nc.dram_tensor("out", [HPC, L - 1], F32, kind="ExternalOutput")

    with tile.TileContext(nc) as tc:
        with (
            tc.tile_pool(name="pp", bufs=1) as pp,
            tc.tile_pool(name="wp", bufs=2) as wp,
            tc.tile_pool(name="psA", bufs=5, space="PSUM") as psum,
            tc.tile_pool(name="psB", bufs=1, space="PSUM") as psum2,
            tc.tile_pool(name="psC", bufs=2, space="PSUM") as psum3,
        ):
            ident = pp.tile([128, 128], F32, tag="ident")
            make_identity(nc, ident[:])
            x2T = pp.tile([128, 8, L], F32, tag="x2T")

            def compute_pre(pre, wih, src, nkt, n_lo, n_hi, rev=False):
                for nt in range(n_lo, n_hi):
                    zp = psum.tile([128, L], F32, tag="ps")
                    for kt in range(nkt):
                        rhs = src[:, kt, ::-1] if rev else src[:, kt, :]
                        nc.tensor.matmul(
                            zp[:], wih[:, kt, (nt - n_lo) * 128:(nt - n_lo + 1) * 128],
                            rhs, start=(kt == 0), stop=(kt == nkt - 1))
                    nc.scalar.activation(pre[:, nt, :], zp[:], AF.Copy)

            with tc.tile_pool(name="pA", bufs=1) as pA:
                x1T = pA.tile([128, 8, L], F32, tag="x1T")

                with tc.tile_pool(name="pB", bufs=1) as pB:
                    pre0 = {d: pB.tile([128, 16, L], F32, tag=f"pre0{d}", name=f"pre0{d}")
                            for d in "fb"}

                    # ---- gather + layer-0 input projections ----
                    with tc.tile_pool(name="pC", bufs=1) as pC, \
                         tc.tile_pool(name="pCw", bufs=2) as pCw:
                        def gather_x(kw, kp, km, tag):
                            widx = pCw.tile([128, 32], I16, tag="widx")
                            pidx = pCw.tile([128, 32], I16, tag="pidx")
                            wpar = pCw.tile([128, 4, 1], F32, tag="wpar")
                            nc.sync.dma_start(widx[:], idx_d[kw][:])
                            nc.sync.dma_start(pidx[:], idx_d[kp][:])
                            nc.sync.dma_start(wpar[:], idx_d[km][:])
                            pair = pCw.tile([128, 4, 2 * WD], F32, tag="pair")
                            nc.gpsimd.dma_gather(pair[:], w2_d[:], widx[:], L, L,
                                                 elem_size=2 * WD)
                            xp = pCw.tile([128, 4, PD], F32, tag="xp")
                            nc.gpsimd.dma_gather(xp[:], pemb_d[:], pidx[:], L, L,
                                                 elem_size=PD)
                            x = pCw.tile([128, 4, DIN], F32, tag="x")
                            xw = x[:, :, 0:WD]
                            nc.vector.tensor_tensor(xw, pair[:, :, WD:2 * WD],
                                                    pair[:, :, 0:WD], OP.subtract)
                            nc.vector.tensor_tensor(
                                xw, xw, wpar[:].to_broadcast([128, 4, WD]), OP.mult)
                            nc.vector.tensor_tensor(xw, xw, pair[:, :, 0:WD], OP.add)
                            nc.vector.tensor_copy(x[:, :, WD:DIN], xp[:])
                            xT = pC.tile([128, 3, L], F32, tag=f"xT{tag}")
                            nc.gpsimd.memset(xT[64:128, 2, :], 0.0)
                            for ct in range(4):
                                for dblk, wdt in ((0, 128), (1, 128), (2, 64)):
                                    tp = psum2.tile([128, 128], F32, tag="ps2")
                                    nc.tensor.transpose(
                                        tp[0:wdt, :],
                                        x[:, ct, dblk * 128:dblk * 128 + wdt],
                                        ident[:])
                                    nc.vector.tensor_copy(
                                        xT[0:wdt, dblk, ct * 128:(ct + 1) * 128],
                                        tp[0:wdt, :])
                            return xT

                        xT = gather_x("w", "p", "m", "f")
                        xTr = gather_x("wr", "pr", "mr", "b")
                        for d, src in (("f", xT), ("b", xTr)):
                            wih = pC.tile([128, 3, NG], F32, tag="wih0")
                            nc.sync.dma_start(wih[:], wih0_d[d][:])
                            compute_pre(pre0[d], wih, src, 3, 0, 16)

                    # ---- layer-0 sweeps ----
                    with tc.tile_pool(name="pD0", bufs=1) as pD0:
                        for d, xdst, xrev in (("f", 0, False), ("b", 4, True)):
                            whh = pD0.tile([128, 4, NG], F32, tag="whh")
                            nc.sync.dma_start(whh[:], whh0_d[d][:])
                            bT = pD0.tile([128, 16], F32, tag="bT")
                            nc.sync.dma_start(bT[:], b0_d[d][:])
                            hfin = _emit_lstm_dir(nc, pD0, wp, psum, pre0[d],
                                                  bT, whh, K0)
                            for kt in range(4):
                                src = hfin[:, kt, ::-1] if xrev else hfin[:, kt, :]
                                nc.vector.tensor_copy(x1T[:, xdst + kt, :], src)

                # ---- layer-1 ----
                with tc.tile_pool(name="pD1", bufs=1) as pD1:
                    pre1 = {d: pD1.tile([128, 16, L], F32, tag=f"pre1{d}", name=f"pre1{d}")
                            for d in "fb"}
                    with tc.tile_pool(name="pE", bufs=1) as pE:
                        x1Tr = pE.tile([128, 8, L], F32, tag="x1Tr")
                        for kt in range(8):
                            nc.vector.tensor_copy(x1Tr[:, kt, :], x1T[:, kt, ::-1])
                        for nh in range(2):   # n-halves of wih1 to save SBUF
                            for d, src in (("f", x1T), ("b", x1Tr)):
                                wih = pE.tile([128, 8, NG // 2], F32, tag="wih1")
                                nc.sync.dma_start(
                                    wih[:], wih1_d[d][:, :, nh * (NG // 2):(nh + 1) * (NG // 2)])
                                compute_pre(pre1[d], wih, src, 8, nh * 8, (nh + 1) * 8)
                    with tc.tile_pool(name="pD1b", bufs=1) as pD1b:
                        for d, xdst, xrev in (("f", 0, False), ("b", 4, True)):
                            whh = pD1b.tile([128, 4, NG], F32, tag="whh")
                            nc.sync.dma_start(whh[:], whh1_d[d][:])
                            bT = pD1b.tile([128, 16], F32, tag="bT")
                            nc.sync.dma_start(bT[:], b1_d[d][:])
                            hfin = _emit_lstm_dir(nc, pD1b, wp, psum, pre1[d],
                                                  bT, whh, K1)
                            for kt in range(4):
                                src = hfin[:, kt, ::-1] if xrev else hfin[:, kt, :]
                                nc.vector.tensor_copy(x2T[:, xdst + kt, :], src)

            # ---- head MLP (both orientations), selection, pairwise ----
            with tc.tile_pool(name="pF", bufs=1) as pF, \
                 tc.tile_pool(name="pFw", bufs=2) as pFw:
                mlpw = pF.tile([128, 8, M], F32, tag="mlpw")
                nc.sync.dma_start(mlpw[:], mlpw_d[:])
                mlp_mt = pF.tile([128, 4, L], F32, tag="mlp_mt")
                mlp_tm = pF.tile([128, 4, M], F32, tag="mlp_tm")
                for mt in range(4):
                    zp = psum.tile([128, L], F32, tag="ps")
                    for kt in range(8):
                        nc.tensor.matmul(zp[:], mlpw[:, kt, mt * 128:(mt + 1) * 128],
                                         x2T[:, kt, :], start=(kt == 0), stop=(kt == 7))
                    nc.scalar.activation(mlp_mt[:, mt, :], zp[:], AF.Copy)
                for tt in range(4):
                    zp = psum.tile([128, M], F32, tag="ps")
                    for kt in range(8):
                        nc.tensor.matmul(zp[:], x2T[:, kt, tt * 128:(tt + 1) * 128],
                                         mlpw[:, kt, :], start=(kt == 0), stop=(kt == 7))
                    nc.scalar.activation(mlp_tm[:, tt, :], zp[:], AF.Copy)

                sel = pF.tile([128, 4, HPC], F32, tag="sel")
                nc.sync.dma_start(sel[:], sel_d[:])
                mlpb2 = pF.tile([128, 4], F32, tag="mlpb2")
                nc.sync.dma_start(mlpb2[:], mlpb2_d[:])
                myb = pF.tile([128, 4, HPC], F32, tag="myb")
                for mt in range(4):
                    zp = psum2.tile([128, HPC], F32, tag="ps2")
                    for tt in range(4):
                        nc.tensor.matmul(zp[:], mlp_tm[:, tt, mt * 128:(mt + 1) * 128],
                                         sel[:, tt, :], start=(tt == 0), stop=(tt == 3))
                    nc.scalar.activation(myb[:, mt, :], zp[:], AF.Identity,
                                         bias=mlpb2[:, mt:mt + 1])

                outw = pF.tile([128, 4], F32, tag="outw")
                nc.sync.dma_start(outw[:], outw_d[:])
                outb = pF.tile([1, 1], F32, tag="outb")
                nc.sync.dma_start(outb[:], outb_d[:])
                for j in range(HPC):
                    sp = psum3.tile([1, L - 1], F32, tag="sp")
                    for mt in range(4):
                        tt_ = pFw.tile([128, L - 1], F32, tag="T")
                        nc.scalar.activation(tt_[:], mlp_mt[:, mt, 1:L], AF.Tanh,
                                             bias=myb[:, mt, j:j + 1])
                        nc.tensor.matmul(sp[:], outw[:, mt:mt + 1], tt_[:],
                                         start=(mt == 0), stop=(mt == 3))
                    srow = pFw.tile([1, L - 1], F32, tag="srow")
                    nc.vector.tensor_tensor(
                        srow[:], sp[:], outb[:].to_broadcast([1, L - 1]), OP.add)
                    nc.sync.dma_start(out_d[j:j + 1, :], srow[:])

    nc.compile()
    return nc


def _packT(W, ktiles, pad_to=None):
    """W [n, k] -> lhsT tile layout [128, ktiles, n]: out[p,kt,n]=W[n,kt*128+p]."""
    WT = np.ascontiguousarray(np.asarray(W).T.astype(np.float32))
    k, n = WT.shape
    if pad_to is not None and k < pad_to:
        WT = np.vstack([WT, np.zeros((pad_to - k, n), np.float32)])
    return np.ascontiguousarray(WT.reshape(ktiles, 128, n).transpose(1, 0, 2))


def _wrap16(idx):
    a = np.asarray(idx).astype(np.int64).reshape(32, 16).T.astype(np.int16)
    return np.ascontiguousarray(np.tile(a, (8, 1)))


def _bpack(b, tiles):
    return np.ascontiguousarray(np.asarray(b, np.float32).reshape(tiles, 128).T)


def kernel(**inputs):
    if "nc" not in _CACHE:
        _CACHE["nc"] = _build_program()
    nc = _CACHE["nc"]

    inp = {k: np.asarray(v) for k, v in inputs.items()}
    widx = inp["word_idx"].astype(np.int64)
    pidx = inp["pos_idx"].astype(np.int64)

    common = {
        "w2": np.ascontiguousarray(
            inp["w_embed"].astype(np.float32).reshape(V2, 2 * WD)),
        "pemb": np.ascontiguousarray(inp["p_embed"].astype(np.float32)),
        "mlpw": _packT(inp["mlp_W"], 8),
        "mlpb2": _bpack(2.0 * inp["mlp_b"], 4),
        "outw": _bpack(inp["out_w"], 4),
        "outb": np.asarray(inp["out_b"], np.float32).reshape(1, 1),
    }
    for sfx, w, p in (("", widx, pidx), ("r", widx[::-1], pidx[::-1])):
        common[f"widx{sfx}"] = _wrap16(w // 2)
        common[f"pidx{sfx}"] = _wrap16(p)
        common[f"wpar{sfx}"] = np.ascontiguousarray(
            (w % 2).astype(np.float32).reshape(4, 128).T.reshape(128, 4, 1))
    for d, sfx in (("f", ""), ("b", "r")):
        common[f"wih0{d}"] = _packT(inp[f"W_ih_l0{sfx}"], 3, pad_to=384)
        common[f"whh0{d}"] = _packT(inp[f"W_hh_l0{sfx}"], 4)
        common[f"b0{d}"] = _bpack(inp[f"b_ih_l0{sfx}"] + inp[f"b_hh_l0{sfx}"], 16)
        common[f"wih1{d}"] = _packT(inp[f"W_ih_l1{sfx}"], 8)
        common[f"whh1{d}"] = _packT(inp[f"W_hh_l1{sfx}"], 4)
        common[f"b1{d}"] = _bpack(inp[f"b_ih_l1{sfx}"] + inp[f"b_hh_l1{sfx}"], 16)

    in_maps = []
    for c in range(NCORES):
        m = dict(common)
        sel = np.zeros((L, HPC), np.float32)
        for j in range(HPC):
            sel[c * HPC + j, j] = 1.0
        m["sel"] = np.ascontiguousarray(sel.reshape(4, 128, HPC).transpose(1, 0, 2))
        in_maps.append(m)

    res = run_bass_kernel_spmd(nc, in_maps, list(range(NCORES)))
    out = np.concatenate([res.results[c]["out"] for c in range(NCORES)], axis=0)
    return out.astype(np.float32)


# revision 11
# speedup vs baseline: 2.1690x; 2.1690x over previous
"""Trainium2 Bass kernel for nn_DependencyParser.

Pipeline (per NeuronCore, SPMD on 8 cores):
  - on-device embedding gather (dma_gather; paired-row trick since the 50000
    vocab exceeds int16 index range)
  - 2-layer BiLSTM solved by Jacobi fixed-point iteration over the hidden
    sequence: each sweep is a batched [2048,512]x[512,512] matmul + gate
    activations; the cell state c is computed EXACTLY per sweep with the
    hardware linear-scan op (tensor_tensor_scan).  The iteration contracts
    at ~0.55x per sweep; K=12 sweeps/layer -> ~1e-3 relative score error.
  - head MLP, then pairwise scores tanh(mlp[i]+mlp[j+1]) @ out_w + out_b,
    sharded over heads: core c computes score rows [64c, 64c+64).
All sequence tensors are in [feature, time] layout (feature on partitions).
Matmul operands use float32r (TF32-like): 2x TensorE stream rate vs fp32
with ~1e-4 rounding.  PRE (input projections + bias) is injected into the
gate PSUM via an identity matmul, keeping the DVE off the critical path;
gate activations are bias-free so two PSUM banks are activated per ACT op.
"""
import sys
sys.path.insert(0, '/opt/trn_rl_repo')
import numpy as np

import concourse.bass as bass
import concourse.mybir as mybir
import concourse.tile as tile
from concourse import bacc
from concourse.masks import make_identity
from concourse.bass_utils import run_bass_kernel_spmd

F32 = mybir.dt.float32
F32R = mybir.dt.float32r
BF16 = mybir.dt.bfloat16
I16 = mybir.dt.int16
AF = mybir.ActivationFunctionType
OP = mybir.AluOpType

L = 512          # sequence length
NG = 2048        # gate width 4*H
V2 = 25000       # paired vocab rows
WD, PD = 256, 64
DIN = WD + PD
M = 512          # mlp width
NCORES = 8
HPC = L // NCORES  # heads per core
K0 = 12          # jacobi sweeps, layer 0
K1 = 12          # jacobi sweeps, layer 1

_CACHE = {}


def _emit_lstm_dir(nc, sp_, wp, gp, psum, identr, zsrc, pre, whh, n_sweeps):
    """Jacobi-iterate one LSTM direction; returns [128, 4, 512] f32r hidden.

    pre: [128, 16, 512] f32r W_ih@x^T + b, n-tile major. whh: [128,4,2048] f32r.
    h buffers store h shifted one step (col t holds h_{t-1}); the final sweep
    writes unshifted output.  Gate tiles are filled two-at-a-time ([128,1024]
    across two PSUM banks) by a single bias-free activation op.
    """
    sig, tanh = AF.Sigmoid, AF.Tanh
    hA = sp_.tile([128, 4, L], F32R, tag="hA")
    hB = sp_.tile([128, 4, L], F32R, tag="hB")
    nc.vector.tensor_copy(hA[:, :, 0:1], zsrc[:, 0:1].to_broadcast([128, 4, 1]))
    nc.vector.tensor_copy(hB[:, :, 0:1], zsrc[:, 0:1].to_broadcast([128, 4, 1]))
    G = {}
    for g in range(4):
        G[g] = gp.tile([128, 4, L], F32, tag=f"G{g}", name=f"G{g}")
    for s in range(n_sweeps):
        hprev, hnew = (hA, hB) if s % 2 == 0 else (hB, hA)
        last = s == n_sweeps - 1
        for a in range(8):          # n-tile pairs (2a, 2a+1), same gate
            gate = (2 * a) // 4
            func = tanh if gate == 2 else sig
            dst = G[gate][:, (2 * a) % 4:(2 * a) % 4 + 2, :]
            if s == 0:
                nc.scalar.activation(dst, pre[:, 2 * a:2 * a + 2, :], func)
            else:
                zp = psum.tile([128, 2, L], F32, tag="zp")
                for half in range(2):
                    nt = 2 * a + half
                    zph = zp[:, half, :]
                    for kt in range(4):
                        nc.tensor.matmul(zph, whh[:, kt, nt * 128:(nt + 1) * 128],
                                         hprev[:, kt, :], start=(kt == 0),
                                         stop=False)
                    nc.tensor.matmul(zph, identr[:], pre[:, nt, :],
                                     start=False, stop=True)
                nc.scalar.activation(dst, zp[:], func)
        for ht in range(4):
            u = wp.tile([128, L], F32, tag="u")
            nc.vector.tensor_tensor(u[:], G[0][:, ht, :], G[2][:, ht, :], OP.mult)
            c = wp.tile([128, L], F32, tag="c")
            nc.vector.tensor_tensor_scan(c[:], G[1][:, ht, :], u[:], 0.0,
                                         OP.mult, OP.add)
            tc_ = wp.tile([128, L], F32, tag="tc")
            nc.scalar.activation(tc_[:], c[:], tanh)
            if last:
                nc.vector.tensor_tensor(hnew[:, ht, :], G[3][:, ht, :], tc_[:],
                                        OP.mult)
            else:
                nc.vector.tensor_tensor(hnew[:, ht, 1:L], G[3][:, ht, 0:L - 1],
                                        tc_[:, 0:L - 1], OP.mult)
    return hA if n_sweeps % 2 == 0 else hB


def _build_program():
    nc = bacc.Bacc("TRN2", target_bir_lowering=False, debug=False,
                   num_devices=NCORES)

    def dram_in(name, shape, dtype=F32):
        return nc.dram_tensor(name, shape, dtype, kind="ExternalInput")

    w2_d = dram_in("w2", [V2, 2 * WD])
    pemb_d = dram_in("pemb", [50, PD])
    idx_d = {}
    for sfx in ("", "r"):
        idx_d["w" + sfx] = dram_in(f"widx{sfx}", [128, 32], I16)
        idx_d["p" + sfx] = dram_in(f"pidx{sfx}", [128, 32], I16)
        idx_d["m" + sfx] = dram_in(f"wpar{sfx}", [128, 4, 1])
    wih0_d = {d: dram_in(f"wih0{d}", [128, 3, NG], F32R) for d in "fb"}
    whh0_d = {d: dram_in(f"whh0{d}", [128, 4, NG], F32R) for d in "fb"}
    b0_d = {d: dram_in(f"b0{d}", [128, 16]) for d in "fb"}
    wih1_d = {d: dram_in(f"wih1{d}", [128, 8, NG], F32R) for d in "fb"}
    whh1_d = {d: dram_in(f"whh1{d}", [128, 4, NG], F32R) for d in "fb"}
    b1_d = {d: dram_in(f"b1{d}", [128, 16]) for d in "fb"}
    mlpw_d = dram_in("mlpw", [128, 8, M], F32R)
    mlpb2_d = dram_in("mlpb2", [128, 4])
    outw_d = dram_in("outw", [128, 4, 2], F32R)
    outb_d = dram_in("outb", [1, 1])
    sel_d = dram_in("sel", [128, 4, HPC], F32R)
    out_d = nc.dram_tensor("out", [HPC, L - 1], F32, kind="ExternalOutput")

    with tile.TileContext(nc) as tc:
        with (
            tc.tile_pool(name="pp", bufs=1) as pp,
            tc.tile_pool(name="wp", bufs=2) as wp,
            tc.tile_pool(name="gp", bufs=1) as gp,
            tc.tile_pool(name="psA", bufs=3, space="PSUM") as psum,
            tc.tile_pool(name="psB", bufs=1, space="PSUM") as psum2,
            tc.tile_pool(name="psC", bufs=1, space="PSUM") as psum3,
        ):
            ident = pp.tile([128, 128], F32, tag="ident")
            make_identity(nc, ident[:])
            identr = pp.tile([128, 128], BF16, tag="identr")
            nc.vector.tensor_copy(identr[:], ident[:])
            zsrc = pp.tile([128, 1], F32, tag="zsrc")
            nc.vector.memset(zsrc[:], 0.0)
            x2T = pp.tile([128, 8, L], F32R, tag="x2T")

            def compute_pre(pre, wih, src, bT, nkt, n_lo, n_hi):
                for nt in range(n_lo, n_hi):
                    zp = psum.tile([128, 2, L], F32, tag="zp")
                    zph = zp[:, 0, :]
                    for kt in range(nkt):
                        nc.tensor.matmul(
                            zph, wih[:, kt, (nt - n_lo) * 128:(nt - n_lo + 1) * 128],
                            src[:, kt, :], start=(kt == 0), stop=(kt == nkt - 1))
                    nc.scalar.activation(pre[:, nt, :], zph, AF.Identity,
                                         bias=bT[:, nt:nt + 1])

            with tc.tile_pool(name="pA", bufs=1) as pA:
                x1T = pA.tile([128, 8, L], F32R, tag="x1T")

                with tc.tile_pool(name="pB", bufs=1) as pB:
                    pre0 = {d: pB.tile([128, 16, L], BF16, tag=f"pre0{d}",
                                       name=f"pre0{d}") for d in "fb"}

                    # ---- gather + layer-0 input projections ----
                    with tc.tile_pool(name="pC", bufs=1) as pC, \
                         tc.tile_pool(name="pCw", bufs=2) as pCw:
                        def gather_x(kw, kp, km, tag):
                            widx = pCw.tile([128, 32], I16, tag="widx")
                            pidx = pCw.tile([128, 32], I16, tag="pidx")
                            wpar = pCw.tile([128, 4, 1], F32, tag="wpar")
                            nc.sync.dma_start(widx[:], idx_d[kw][:])
                            nc.sync.dma_start(pidx[:], idx_d[kp][:])
                            nc.sync.dma_start(wpar[:], idx_d[km][:])
                            pair = pCw.tile([128, 4, 2 * WD], F32, tag="pair")
                            nc.gpsimd.dma_gather(pair[:], w2_d[:], widx[:], L, L,
                                                 elem_size=2 * WD)
                            xp = pCw.tile([128, 4, PD], F32, tag="xp")
                            nc.gpsimd.dma_gather(xp[:], pemb_d[:], pidx[:], L, L,
                                                 elem_size=PD)
                            x = pCw.tile([128, 4, DIN], F32, tag="x")
                            xw = x[:, :, 0:WD]
                            nc.vector.tensor_tensor(xw, pair[:, :, WD:2 * WD],
                                                    pair[:, :, 0:WD], OP.subtract)
                            nc.vector.tensor_tensor(
                                xw, xw, wpar[:].to_broadcast([128, 4, WD]), OP.mult)
                            nc.vector.tensor_tensor(xw, xw, pair[:, :, 0:WD], OP.add)
                            nc.vector.tensor_copy(x[:, :, WD:DIN], xp[:])
                            xT = pC.tile([128, 3, L], F32R, tag=f"xT{tag}",
                                         name=f"xT{tag}")
                            nc.vector.tensor_copy(xT[64:128, 2, :], zsrc[64:128, 0:1].to_broadcast([64, L]))
                            for ct in range(4):
                                for dblk, wdt in ((0, 128), (1, 128), (2, 64)):
                                    tp = psum2.tile([128, 128], F32, tag="ps2")
                                    nc.tensor.transpose(
                                        tp[0:wdt, :],
                                        x[:, ct, dblk * 128:dblk * 128 + wdt],
                                        ident[:])
                                    nc.vector.tensor_copy(
                                        xT[0:wdt, dblk, ct * 128:(ct + 1) * 128],
                                        tp[0:wdt, :])
                            return xT

                        xT = gather_x("w", "p", "m", "f")
                        xTr = gather_x("wr", "pr", "mr", "b")
                        bT0 = {}
                        for d in "fb":
                            bT0[d] = pC.tile([128, 16], F32, tag=f"bT0{d}",
                                             name=f"bT0{d}")
                            nc.sync.dma_start(bT0[d][:], b0_d[d][:])
                        for nh in range(2):
                            for d, src in (("f", xT), ("b", xTr)):
                                wih = pC.tile([128, 3, NG // 2], F32R, tag="wih0")
                                nc.sync.dma_start(
                                    wih[:],
                                    wih0_d[d][:, :, nh * (NG // 2):(nh + 1) * (NG // 2)])
                                compute_pre(pre0[d], wih, src, bT0[d], 3,
                                            nh * 8, (nh + 1) * 8)

                    # ---- layer-0 sweeps ----
                    with tc.tile_pool(name="pD0", bufs=1) as pD0:
                        for d, xdst, xrev in (("f", 0, False), ("b", 4, True)):
                            whh = pD0.tile([128, 4, NG], F32R, tag="whh")
                            nc.sync.dma_start(whh[:], whh0_d[d][:])
                            hfin = _emit_lstm_dir(nc, pD0, wp, gp, psum, identr,
                                                  zsrc, pre0[d], whh, K0)
                            for kt in range(4):
                                src = hfin[:, kt, ::-1] if xrev else hfin[:, kt, :]
                                nc.vector.tensor_copy(x1T[:, xdst + kt, :], src)

                # ---- layer-1 ----
                with tc.tile_pool(name="pD1", bufs=1) as pD1:
                    pre1 = {d: pD1.tile([128, 16, L], BF16, tag=f"pre1{d}",
                                        name=f"pre1{d}") for d in "fb"}
                    with tc.tile_pool(name="pE", bufs=1) as pE:
                        x1Tr = pE.tile([128, 8, L], F32R, tag="x1Tr")
                        for kt in range(8):
                            nc.vector.tensor_copy(x1Tr[:, kt, :], x1T[:, kt, ::-1])
                        bT1 = {}
                        for d in "fb":
                            bT1[d] = pE.tile([128, 16], F32, tag=f"bT1{d}",
                                             name=f"bT1{d}")
                            nc.sync.dma_start(bT1[d][:], b1_d[d][:])
                        for nh in range(2):   # n-halves of wih1 to save SBUF
                            for d, src in (("f", x1T), ("b", x1Tr)):
                                wih = pE.tile([128, 8, NG // 2], F32R, tag="wih1")
                                nc.sync.dma_start(
                                    wih[:],
                                    wih1_d[d][:, :, nh * (NG // 2):(nh + 1) * (NG // 2)])
                                compute_pre(pre1[d], wih, src, bT1[d], 8,
                                            nh * 8, (nh + 1) * 8)
                    with tc.tile_pool(name="pD1b", bufs=1) as pD1b:
                        for d, xdst, xrev in (("f", 0, False), ("b", 4, True)):
                            whh = pD1b.tile([128, 4, NG], F32R, tag="whh")
                            nc.sync.dma_start(whh[:], whh1_d[d][:])
                            hfin = _emit_lstm_dir(nc, pD1b, wp, gp, psum, identr,
                                                  zsrc, pre1[d], whh, K1)
                            for kt in range(4):
                                src = hfin[:, kt, ::-1] if xrev else hfin[:, kt, :]
                                nc.vector.tensor_copy(x2T[:, xdst + kt, :], src)

            # ---- head MLP (both orientations), selection, pairwise ----
            with tc.tile_pool(name="pF", bufs=1) as pF, \
                 tc.tile_pool(name="pFw", bufs=2) as pFw:
                mlpw = pF.tile([128, 8, M], F32R, tag="mlpw")
                nc.sync.dma_start(mlpw[:], mlpw_d[:])
                mlp_mt = pF.tile([128, 4, L], F32R, tag="mlp_mt")
                mlp_tm = pF.tile([128, 4, M], F32R, tag="mlp_tm")
                for mt in range(4):
                    zp = psum.tile([128, 2, L], F32, tag="zp")
                    zph = zp[:, 0, :]
                    for kt in range(8):
                        nc.tensor.matmul(zph, mlpw[:, kt, mt * 128:(mt + 1) * 128],
                                         x2T[:, kt, :], start=(kt == 0), stop=(kt == 7))
                    nc.scalar.activation(mlp_mt[:, mt, :], zph, AF.Copy)
                for tt in range(4):
                    zp = psum.tile([128, 2, L], F32, tag="zp")
                    zph = zp[:, 0, :]
                    for kt in range(8):
                        nc.tensor.matmul(zph, x2T[:, kt, tt * 128:(tt + 1) * 128],
                                         mlpw[:, kt, :], start=(kt == 0), stop=(kt == 7))
                    nc.scalar.activation(mlp_tm[:, tt, :], zph, AF.Copy)

                sel = pF.tile([128, 4, HPC], F32R, tag="sel")
                nc.sync.dma_start(sel[:], sel_d[:])
                mlpb2 = pF.tile([128, 4], F32, tag="mlpb2")
                nc.sync.dma_start(mlpb2[:], mlpb2_d[:])
                myb = pF.tile([128, 4, HPC], F32, tag="myb")
                for mt in range(4):
                    zp = psum2.tile([128, HPC], F32, tag="ps2")
                    for tt in range(4):
                        nc.tensor.matmul(zp[:], mlp_tm[:, tt, mt * 128:(mt + 1) * 128],
                                         sel[:, tt, :], start=(tt == 0), stop=(tt == 3))
                    nc.scalar.activation(myb[:, mt, :], zp[:], AF.Identity,
                                         bias=mlpb2[:, mt:mt + 1])

                outw = pF.tile([128, 4, 2], F32R, tag="outw")
                nc.sync.dma_start(outw[:], outw_d[:])
                outb = pF.tile([1, 1], F32, tag="outb")
                nc.sync.dma_start(outb[:], outb_d[:])
                for j in range(HPC):
                    sp = psum3.tile([2, L], F32, tag="sp")
                    for mt in range(4):
                        tt_ = pFw.tile([128, L], F32R, tag="T")
                        nc.scalar.activation(tt_[:], mlp_mt[:, mt, :], AF.Tanh,
                                             bias=myb[:, mt, j:j + 1])
                        nc.tensor.matmul(sp[:], outw[:, mt, :], tt_[:],
                                         start=(mt == 0), stop=(mt == 3))
                    srow = pFw.tile([1, L - 1], F32, tag="srow")
                    nc.vector.tensor_tensor(
                        srow[:], sp[0:1, 1:L], outb[:].to_broadcast([1, L - 1]),
                        OP.add)
                    nc.sync.dma_start(out_d[j:j + 1, :], srow[:])

    nc.compile()
    return nc


def _packT(W, ktiles, pad_to=None):
    """W [n, k] -> lhsT tile layout [128, ktiles, n]: out[p,kt,n]=W[n,kt*128+p]."""
    WT = np.ascontiguousarray(np.asarray(W).T.astype(np.float32))
    k, n = WT.shape
    if pad_to is not None and k < pad_to:
        WT = np.vstack([WT, np.zeros((pad_to - k, n), np.float32)])
    return np.ascontiguousarray(WT.reshape(ktiles, 128, n).transpose(1, 0, 2))


def _wrap16(idx):
    a = np.asarray(idx).astype(np.int64).reshape(32, 16).T.astype(np.int16)
    return np.ascontiguousarray(np.tile(a, (8, 1)))


def _bpack(b, tiles):
    return np.ascontiguousarray(np.asarray(b, np.float32).reshape(tiles, 128).T)


def kernel(**inputs):
    if "nc" not in _CACHE:
        _CACHE["nc"] = _build_program()
    nc = _CACHE["nc"]

    inp = {k: np.asarray(v) for k, v in inputs.items()}
    widx = inp["word_idx"].astype(np.int64)
    pidx = inp["pos_idx"].astype(np.int64)

    common = {
        "w2": np.ascontiguousarray(
            inp["w_embed"].astype(np.float32).reshape(V2, 2 * WD)),
        "pemb": np.ascontiguousarray(inp["p_embed"].astype(np.float32)),
        "mlpw": _packT(inp["mlp_W"], 8),
        "mlpb2": _bpack(2.0 * inp["mlp_b"], 4),
        "outw": np.ascontiguousarray(np.stack([_bpack(inp["out_w"], 4), np.zeros((128, 4), np.float32)], axis=-1)),
        "outb": np.asarray(inp["out_b"], np.float32).reshape(1, 1),
    }
    for sfx, w, p in (("", widx, pidx), ("r", widx[::-1], pidx[::-1])):
        common[f"widx{sfx}"] = _wrap16(w // 2)
        common[f"pidx{sfx}"] = _wrap16(p)
        common[f"wpar{sfx}"] = np.ascontiguousarray(
            (w % 2).astype(np.float32).reshape(4, 128).T.reshape(128, 4, 1))
    for d, sfx in (("f", ""), ("b", "r")):
        common[f"wih0{d}"] = _packT(inp[f"W_ih_l0{sfx}"], 3, pad_to=384)
        common[f"whh0{d}"] = _packT(inp[f"W_hh_l0{sfx}"], 4)
        common[f"b0{d}"] = _bpack(inp[f"b_ih_l0{sfx}"] + inp[f"b_hh_l0{sfx}"], 16)
        common[f"wih1{d}"] = _packT(inp[f"W_ih_l1{sfx}"], 8)
        common[f"whh1{d}"] = _packT(inp[f"W_hh_l1{sfx}"], 4)
        common[f"b1{d}"] = _bpack(inp[f"b_ih_l1{sfx}"] + inp[f"b_hh_l1{sfx}"], 16)

    in_maps = []
    for c in range(NCORES):
        m = dict(common)
        sel = np.zeros((L, HPC), np.float32)
        for j in range(HPC):
            sel[c * HPC + j, j] = 1.0
        m["sel"] = np.ascontiguousarray(sel.reshape(4, 128, HPC).transpose(1, 0, 2))
        in_maps.append(m)

    res = run_bass_kernel_spmd(nc, in_maps, list(range(NCORES)))
    out = np.concatenate([res.results[c]["out"] for c in range(NCORES)], axis=0)
    return out.astype(np.float32)


# revision 13
# speedup vs baseline: 3.0736x; 1.4170x over previous
"""Trainium2 Bass kernel for nn_DependencyParser.

SPMD over 8 NeuronCores; cores 0-3 run the forward LSTM direction, cores 4-7
the backward direction -- one identical program, direction expressed purely
through per-core DATA (time-reversed gather indices, direction-specific
weights, 0/1 orientation masks).  Per core:
  - on-device embedding gather (dma_gather; paired-row trick since the 50000
    vocab exceeds the int16 index range), PE-transpose to [feature, time]
  - 2-layer LSTM via Jacobi fixed-point iteration over the hidden sequence:
    each sweep is a batched [2048,512]x[512,512] matmul + gate activations;
    the cell state is computed EXACTLY per sweep with the hardware linear
    scan op (tensor_tensor_scan).  Contraction ~0.55x/sweep; K=10 sweeps.
  - after each layer the two directions exchange hidden states with a
    pairwise AllGather; each core rebuilds the bidirectional input in its
    own time orientation with 0/1 mask combines (reversal = negative-stride
    DVE copies).
  - head MLP, then pairwise scores tanh(mlp[i]+mlp[j+1]) @ out_w + out_b for
    this core's 64 head rows (head selection via a one-hot matmul).
Matmul operands are float32r (TF32-like: 2x stream rate of fp32, ~1e-4
rounding); PRE = W_ih@x + b is stored bf16 and injected into the gate PSUM
via identity-matmuls and DVE adds (split to balance the two engines).
"""
import sys
sys.path.insert(0, '/opt/trn_rl_repo')
import numpy as np

import concourse.bass as bass
import concourse.mybir as mybir
import concourse.tile as tile
from concourse import bacc
from concourse.masks import make_identity
from concourse.bass_utils import run_bass_kernel_spmd

F32 = mybir.dt.float32
F32R = mybir.dt.float32r
BF16 = mybir.dt.bfloat16
I16 = mybir.dt.int16
AF = mybir.ActivationFunctionType
OP = mybir.AluOpType

L = 512
NG = 2048
V2 = 25000
WD, PD = 256, 64
DIN = WD + PD
M = 512
NCORES = 8
HPC = L // NCORES
K0 = 10
K1 = 10
PAIR_ORDER = (0, 2, 4, 6, 1, 3, 5, 7)   # i,f,g,o for h0/h1 first, then h2/h3
N_TE_INJECT = 4                          # pre-inject via TensorE for last halves

_CACHE = {}


def _emit_lstm_dir(nc, sp_, wp, gp, psum, identr, zsrc, pre, whh, n_sweeps):
    """Jacobi-iterate one LSTM direction; returns [128, 4, 512] f32r hidden."""
    sig, tanh = AF.Sigmoid, AF.Tanh
    hA = sp_.tile([128, 4, L], F32R, tag="hA")
    hB = sp_.tile([128, 4, L], F32R, tag="hB")
    nc.vector.tensor_copy(hA[:, :, 0:1], zsrc[:, 0:1].to_broadcast([128, 4, 1]))
    nc.vector.tensor_copy(hB[:, :, 0:1], zsrc[:, 0:1].to_broadcast([128, 4, 1]))
    G = {}
    for g in range(4):
        G[g] = gp.tile([128, 4, L], F32, tag=f"G{g}", name=f"G{g}")
    for s in range(n_sweeps):
        hprev, hnew = (hA, hB) if s % 2 == 0 else (hB, hA)
        last = s == n_sweeps - 1
        for a in PAIR_ORDER:        # n-tile pair (2a, 2a+1), same gate
            gate = (2 * a) // 4
            func = tanh if gate == 2 else sig
            dst = G[gate][:, (2 * a) % 4:(2 * a) % 4 + 2, :]
            if s == 0:
                nc.scalar.activation(dst, pre[:, 2 * a:2 * a + 2, :], func)
            else:
                zp = psum.tile([128, 2, L], F32, tag="zp")
                for half in range(2):
                    nt = 2 * a + half
                    zph = zp[:, half, :]
                    te_inject = nt >= 16 - N_TE_INJECT
                    for kt in range(4):
                        nc.tensor.matmul(zph, whh[:, kt, nt * 128:(nt + 1) * 128],
                                         hprev[:, kt, :], start=(kt == 0),
                                         stop=(kt == 3 and not te_inject))
                    if te_inject:
                        nc.tensor.matmul(zph, identr[:], pre[:, nt, :],
                                         start=False, stop=True)
                    else:
                        nc.vector.tensor_tensor(zph, zph, pre[:, nt, :], OP.add)
                nc.scalar.activation(dst, zp[:], func)
        for ht in range(4):
            u = wp.tile([128, L], F32, tag="u")
            nc.vector.tensor_tensor(u[:], G[0][:, ht, :], G[2][:, ht, :], OP.mult)
            c = wp.tile([128, L], F32, tag="c")
            nc.vector.tensor_tensor_scan(c[:], G[1][:, ht, :], u[:], 0.0,
                                         OP.mult, OP.add)
            tc_ = wp.tile([128, L], F32, tag="tc")
            nc.scalar.activation(tc_[:], c[:], tanh)
            if last:
                nc.vector.tensor_tensor(hnew[:, ht, :], G[3][:, ht, :], tc_[:],
                                        OP.mult)
            else:
                nc.vector.tensor_tensor(hnew[:, ht, 1:L], G[3][:, ht, 0:L - 1],
                                        tc_[:, 0:L - 1], OP.mult)
    return hA if n_sweeps % 2 == 0 else hB


def _build_program():
    nc = bacc.Bacc("TRN2", target_bir_lowering=False, debug=False,
                   num_devices=NCORES)

    def dram_in(name, shape, dtype=F32):
        return nc.dram_tensor(name, shape, dtype, kind="ExternalInput")

    w2_d = dram_in("w2", [V2, 2 * WD])
    pemb_d = dram_in("pemb", [50, PD])
    widx_d = dram_in("widx", [128, 32], I16)
    pidx_d = dram_in("pidx", [128, 32], I16)
    wpar_d = dram_in("wpar", [128, 4, 1])
    wih0_d = dram_in("wih0", [128, 3, NG], F32R)
    whh0_d = dram_in("whh0", [128, 4, NG], F32R)
    b0_d = dram_in("b0", [128, 16])
    wih1_d = dram_in("wih1", [128, 8, NG], F32R)
    whh1_d = dram_in("whh1", [128, 4, NG], F32R)
    b1_d = dram_in("b1", [128, 16])
    mlpw_d = dram_in("mlpw", [128, 8, M], F32R)
    mlpb2_d = dram_in("mlpb2", [128, 4])
    outw_d = dram_in("outw", [128, 4, 2], F32R)
    outb_d = dram_in("outb", [1, 1])
    sel_d = dram_in("sel", [128, 4, HPC], F32R)
    mi_d = dram_in("maskI", [128, 1])
    mr_d = dram_in("maskR", [128, 1])
    out_d = nc.dram_tensor("out", [HPC, L], F32, kind="ExternalOutput")

    GROUPS = [[0, 4], [1, 5], [2, 6], [3, 7]]

    with tile.TileContext(nc) as tc:
        with (
            tc.tile_pool(name="pp", bufs=1) as pp,
            tc.tile_pool(name="wp", bufs=2) as wp,
            tc.tile_pool(name="gp", bufs=1) as gp,
            tc.tile_pool(name="ep", bufs=1) as ep,
            tc.tile_pool(name="dram", bufs=2, space="DRAM") as dp,
            tc.tile_pool(name="psA", bufs=3, space="PSUM") as psum,
            tc.tile_pool(name="psB", bufs=1, space="PSUM") as psum2,
            tc.tile_pool(name="psC", bufs=1, space="PSUM") as psum3,
        ):
            ident = pp.tile([128, 128], F32, tag="ident")
            make_identity(nc, ident[:])
            identr = pp.tile([128, 128], BF16, tag="identr")
            nc.vector.tensor_copy(identr[:], ident[:])
            zsrc = pp.tile([128, 1], F32, tag="zsrc")
            nc.vector.memset(zsrc[:], 0.0)
            mI = pp.tile([128, 1], F32, tag="mI")
            mR = pp.tile([128, 1], F32, tag="mR")
            nc.sync.dma_start(mI[:], mi_d[:])
            nc.sync.dma_start(mR[:], mr_d[:])
            x1T = pp.tile([128, 8, L], F32R, tag="x1T")
            x2T = pp.tile([128, 8, L], F32R, tag="x2T")

            def compute_pre(pre, wih, src, bT, nkt):
                for nt in range(16):
                    zp = psum.tile([128, 2, L], F32, tag="zp")
                    zph = zp[:, 0, :]
                    for kt in range(nkt):
                        nc.tensor.matmul(zph, wih[:, kt, nt * 128:(nt + 1) * 128],
                                         src[:, kt, :], start=(kt == 0),
                                         stop=(kt == nkt - 1))
                    nc.scalar.activation(pre[:, nt, :], zph, AF.Identity,
                                         bias=bT[:, nt:nt + 1])

            def exchange(hfin, xdst):
                """AllGather my hidden with my partner; build [128, 8, L]
                bidirectional input in MY time orientation via 0/1 masks."""
                inb = dp.tile([128, 4, L], F32R, tag="inb")
                outb_t = dp.tile([2, 128, 4, L], F32R, tag="outb_t")
                nc.sync.dma_start(inb[:], hfin[:])
                nc.gpsimd.collective_compute(
                    "AllGather", OP.bypass, replica_groups=GROUPS,
                    ins=[inb.opt()], outs=[outb_t.opt()])
                s0 = ep.tile([128, 4, L], F32R, tag="slot0")
                s1 = ep.tile([128, 4, L], F32R, tag="slot1")
                nc.sync.dma_start(s0[:], outb_t[0])
                nc.sync.dma_start(s1[:], outb_t[1])
                tmp = ep.tile([128, L], F32, tag="xsel")
                for kt in range(4):
                    nc.vector.tensor_tensor(
                        tmp[:], s0[:, kt, :], mI[:].to_broadcast([128, L]), OP.mult)
                    nc.vector.tensor_tensor(
                        xdst[:, kt, 0:L], s0[:, kt, ::-1],
                        mR[:].to_broadcast([128, L]), OP.mult)
                    nc.vector.tensor_tensor(xdst[:, kt, :], xdst[:, kt, :],
                                            tmp[:], OP.add)
                    nc.vector.tensor_tensor(
                        tmp[:], s1[:, kt, ::-1], mI[:].to_broadcast([128, L]),
                        OP.mult)
                    nc.vector.tensor_tensor(
                        xdst[:, 4 + kt, 0:L], s1[:, kt, :],
                        mR[:].to_broadcast([128, L]), OP.mult)
                    nc.vector.tensor_tensor(xdst[:, 4 + kt, :],
                                            xdst[:, 4 + kt, :], tmp[:], OP.add)

            # ---------- gather + transpose + layer 0 ----------
            with tc.tile_pool(name="p0a", bufs=1) as p0a:
              pre0 = p0a.tile([128, 16, L], BF16, tag="pre0")
              bT0 = p0a.tile([128, 16], F32, tag="bT0")
              nc.sync.dma_start(bT0[:], b0_d[:])
              with tc.tile_pool(name="pC", bufs=1) as pC, \
                 tc.tile_pool(name="pCw", bufs=1) as pCw:
                widx = pCw.tile([128, 32], I16, tag="widx")
                pidx = pCw.tile([128, 32], I16, tag="pidx")
                wpar = pCw.tile([128, 4, 1], F32, tag="wpar")
                nc.sync.dma_start(widx[:], widx_d[:])
                nc.sync.dma_start(pidx[:], pidx_d[:])
                nc.sync.dma_start(wpar[:], wpar_d[:])
                pair = pCw.tile([128, 4, 2 * WD], F32, tag="pair")
                nc.gpsimd.dma_gather(pair[:], w2_d[:], widx[:], L, L,
                                     elem_size=2 * WD)
                xp = pCw.tile([128, 4, PD], F32, tag="xp")
                nc.gpsimd.dma_gather(xp[:], pemb_d[:], pidx[:], L, L,
                                     elem_size=PD)
                x = pCw.tile([128, 4, DIN], F32, tag="x")
                xw = x[:, :, 0:WD]
                nc.vector.tensor_tensor(xw, pair[:, :, WD:2 * WD],
                                        pair[:, :, 0:WD], OP.subtract)
                nc.vector.tensor_tensor(xw, xw,
                                        wpar[:].to_broadcast([128, 4, WD]), OP.mult)
                nc.vector.tensor_tensor(xw, xw, pair[:, :, 0:WD], OP.add)
                nc.vector.tensor_copy(x[:, :, WD:DIN], xp[:])
                xT = pC.tile([128, 3, L], F32R, tag="xT")
                nc.vector.tensor_copy(
                    xT[64:128, 2, :], zsrc[64:128, 0:1].to_broadcast([64, L]))
                for ct in range(4):
                    for dblk, wdt in ((0, 128), (1, 128), (2, 64)):
                        tp = psum2.tile([128, 128], F32, tag="ps2")
                        nc.tensor.transpose(
                            tp[0:wdt, :], x[:, ct, dblk * 128:dblk * 128 + wdt],
                            ident[:])
                        nc.vector.tensor_copy(
                            xT[0:wdt, dblk, ct * 128:(ct + 1) * 128], tp[0:wdt, :])

                with tc.tile_pool(name="pw0", bufs=1) as pw0:
                    wih0 = pw0.tile([128, 3, NG], F32R, tag="wih0")
                    nc.sync.dma_start(wih0[:], wih0_d[:])
                    compute_pre(pre0, wih0, xT, bT0, 3)

              with tc.tile_pool(name="p0b", bufs=1) as p0b:
                  whh0 = p0b.tile([128, 4, NG], F32R, tag="whh0")
                  nc.sync.dma_start(whh0[:], whh0_d[:])
                  hfin0 = _emit_lstm_dir(nc, p0b, wp, gp, psum, identr, zsrc,
                                         pre0, whh0, K0)
                  exchange(hfin0, x1T)

            # ---------- layer 1 ----------
            with tc.tile_pool(name="p1a", bufs=1) as p1a:
                pre1 = p1a.tile([128, 16, L], BF16, tag="pre1")
                bT1 = p1a.tile([128, 16], F32, tag="bT1")
                nc.sync.dma_start(bT1[:], b1_d[:])
                with tc.tile_pool(name="p1w", bufs=1) as p1w:
                    for nh in range(2):
                        wih1 = p1w.tile([128, 8, NG // 2], F32R, tag="wih1")
                        nc.sync.dma_start(
                            wih1[:],
                            wih1_d[:, :, nh * (NG // 2):(nh + 1) * (NG // 2)])
                        for nt in range(nh * 8, (nh + 1) * 8):
                            zp = psum.tile([128, 2, L], F32, tag="zp",
                                           name=f"prez{nh}_{nt}")
                            zph = zp[:, 0, :]
                            for kt in range(8):
                                nc.tensor.matmul(
                                    zph,
                                    wih1[:, kt, (nt - nh * 8) * 128:(nt - nh * 8 + 1) * 128],
                                    x1T[:, kt, :], start=(kt == 0), stop=(kt == 7))
                            nc.scalar.activation(pre1[:, nt, :], zph, AF.Identity,
                                                 bias=bT1[:, nt:nt + 1])
                with tc.tile_pool(name="p1b", bufs=1) as p1b:
                    whh1 = p1b.tile([128, 4, NG], F32R, tag="whh1")
                    nc.sync.dma_start(whh1[:], whh1_d[:])
                    hfin1 = _emit_lstm_dir(nc, p1b, wp, gp, psum, identr, zsrc,
                                           pre1, whh1, K1)
                    exchange(hfin1, x2T)

            # ---------- MLP + pairwise ----------
            with tc.tile_pool(name="pF", bufs=1) as pF, \
                 tc.tile_pool(name="pFw", bufs=2) as pFw:
                mlpw = pF.tile([128, 8, M], F32R, tag="mlpw")
                nc.sync.dma_start(mlpw[:], mlpw_d[:])
                mlp_mt = pF.tile([128, 4, L], F32R, tag="mlp_mt")
                mlp_tm = pF.tile([128, 4, M], F32R, tag="mlp_tm")
                for mt in range(4):
                    zp = psum.tile([128, 2, L], F32, tag="zp")
                    zph = zp[:, 0, :]
                    for kt in range(8):
                        nc.tensor.matmul(zph, mlpw[:, kt, mt * 128:(mt + 1) * 128],
                                         x2T[:, kt, :], start=(kt == 0),
                                         stop=(kt == 7))
                    nc.scalar.activation(mlp_mt[:, mt, :], zph, AF.Copy)
                for tt in range(4):
                    zp = psum.tile([128, 2, L], F32, tag="zp")
                    zph = zp[:, 0, :]
                    for kt in range(8):
                        nc.tensor.matmul(zph, x2T[:, kt, tt * 128:(tt + 1) * 128],
                                         mlpw[:, kt, :], start=(kt == 0),
                                         stop=(kt == 7))
                    nc.scalar.activation(mlp_tm[:, tt, :], zph, AF.Copy)

                sel = pF.tile([128, 4, HPC], F32R, tag="sel")
                nc.sync.dma_start(sel[:], sel_d[:])
                mlpb2 = pF.tile([128, 4], F32, tag="mlpb2")
                nc.sync.dma_start(mlpb2[:], mlpb2_d[:])
                myb = pF.tile([128, 4, HPC], F32, tag="myb")
                for mt in range(4):
                    zp = psum2.tile([128, HPC], F32, tag="ps2")
                    for tt in range(4):
                        nc.tensor.matmul(zp[:],
                                         mlp_tm[:, tt, mt * 128:(mt + 1) * 128],
                                         sel[:, tt, :], start=(tt == 0),
                                         stop=(tt == 3))
                    nc.scalar.activation(myb[:, mt, :], zp[:], AF.Identity,
                                         bias=mlpb2[:, mt:mt + 1])

                outw = pF.tile([128, 4, 2], F32R, tag="outw")
                nc.sync.dma_start(outw[:], outw_d[:])
                outb = pF.tile([1, 1], F32, tag="outb")
                nc.sync.dma_start(outb[:], outb_d[:])
                for j in range(HPC):
                    sp = psum3.tile([2, L], F32, tag="sp")
                    for mt in range(4):
                        tt_ = pFw.tile([128, L], F32R, tag="T")
                        nc.scalar.activation(tt_[:], mlp_mt[:, mt, :], AF.Tanh,
                                             bias=myb[:, mt, j:j + 1])
                        nc.tensor.matmul(sp[:], outw[:, mt, :], tt_[:],
                                         start=(mt == 0), stop=(mt == 3))
                    srow = pFw.tile([1, L], F32, tag="srow")
                    nc.vector.tensor_tensor(
                        srow[:], sp[0:1, :], outb[:].to_broadcast([1, L]), OP.add)
                    nc.sync.dma_start(out_d[j:j + 1, :], srow[:])

    nc.compile()
    return nc


def _packT(W, ktiles, pad_to=None):
    WT = np.ascontiguousarray(np.asarray(W).T.astype(np.float32))
    k, n = WT.shape
    if pad_to is not None and k < pad_to:
        WT = np.vstack([WT, np.zeros((pad_to - k, n), np.float32)])
    return np.ascontiguousarray(WT.reshape(ktiles, 128, n).transpose(1, 0, 2))


def _wrap16(idx):
    a = np.asarray(idx).astype(np.int64).reshape(32, 16).T.astype(np.int16)
    return np.ascontiguousarray(np.tile(a, (8, 1)))


def _bpack(b, tiles):
    return np.ascontiguousarray(np.asarray(b, np.float32).reshape(tiles, 128).T)


def kernel(**inputs):
    if "nc" not in _CACHE:
        _CACHE["nc"] = _build_program()
    nc = _CACHE["nc"]

    inp = {k: np.asarray(v) for k, v in inputs.items()}
    widx = inp["word_idx"].astype(np.int64)
    pidx = inp["pos_idx"].astype(np.int64)

    base = {
        "w2": np.ascontiguousarray(
            inp["w_embed"].astype(np.float32).reshape(V2, 2 * WD)),
        "pemb": np.ascontiguousarray(inp["p_embed"].astype(np.float32)),
        "mlpw": _packT(inp["mlp_W"], 8),
        "mlpb2": _bpack(2.0 * inp["mlp_b"], 4),
        "outw": np.ascontiguousarray(
            np.stack([_bpack(inp["out_w"], 4), np.zeros((128, 4), np.float32)],
                     axis=-1)),
        "outb": np.asarray(inp["out_b"], np.float32).reshape(1, 1),
    }

    def dir_inputs(rev):
        w = widx[::-1] if rev else widx
        p = pidx[::-1] if rev else pidx
        sfx = "r" if rev else ""
        return {
            "widx": _wrap16(w // 2),
            "pidx": _wrap16(p),
            "wpar": np.ascontiguousarray(
                (w % 2).astype(np.float32).reshape(4, 128).T.reshape(128, 4, 1)),
            "wih0": _packT(inp[f"W_ih_l0{sfx}"], 3, pad_to=384),
            "whh0": _packT(inp[f"W_hh_l0{sfx}"], 4),
            "b0": _bpack(inp[f"b_ih_l0{sfx}"] + inp[f"b_hh_l0{sfx}"], 16),
            "wih1": _packT(inp[f"W_ih_l1{sfx}"], 8),
            "whh1": _packT(inp[f"W_hh_l1{sfx}"], 4),
            "b1": _bpack(inp[f"b_ih_l1{sfx}"] + inp[f"b_hh_l1{sfx}"], 16),
            "maskI": np.full((128, 1), 0.0 if rev else 1.0, np.float32),
            "maskR": np.full((128, 1), 1.0 if rev else 0.0, np.float32),
        }

    fwd_in, bwd_in = dir_inputs(False), dir_inputs(True)

    in_maps = []
    for c in range(NCORES):
        rev = c >= 4
        m = dict(base)
        m.update(bwd_in if rev else fwd_in)
        sel = np.zeros((L, HPC), np.float32)
        for j in range(HPC):
            i = c * HPC + j                        # global head row
            t = (L - 1 - i) if rev else i          # row in this core's time
            sel[t, j] = 1.0
        m["sel"] = np.ascontiguousarray(
            sel.reshape(4, 128, HPC).transpose(1, 0, 2))
        in_maps.append(m)

    res = run_bass_kernel_spmd(nc, in_maps, list(range(NCORES)))
    out = np.empty((L, L - 1), np.float32)
    for c in range(NCORES):
        o = res.results[c]["out"]                  # [HPC, 512] in core time
        for j in range(HPC):
            i = c * HPC + j
            if c >= 4:
                out[i] = o[j, 0:L - 1][::-1]
            else:
                out[i] = o[j, 1:L]
    return out


# revision 14
# speedup vs baseline: 3.8924x; 1.2664x over previous
"""Trainium2 Bass kernel for nn_DependencyParser.

SPMD over 8 NeuronCores; cores 0-3 run the forward LSTM direction, cores 4-7
the backward direction -- one identical program, direction expressed purely
through per-core DATA (time-reversed gather indices, direction-specific
weights, 0/1 orientation masks).  Per core:
  - on-device embedding gather (dma_gather; paired-row trick since the 50000
    vocab exceeds the int16 index range), PE-transpose to [feature, time]
  - 2-layer LSTM via Jacobi fixed-point iteration over the hidden sequence:
    each sweep is a batched [2048,512]x[512,512] matmul + gate activations;
    the cell state is computed EXACTLY per sweep with the hardware linear
    scan op (tensor_tensor_scan).  Contraction ~0.55x/sweep; K=10 sweeps.
  - after each layer the two directions exchange hidden states with a
    pairwise AllGather; each core rebuilds the bidirectional input in its
    own time orientation with 0/1 mask combines (reversal = negative-stride
    DVE copies).
  - head MLP, then pairwise scores tanh(mlp[i]+mlp[j+1]) @ out_w + out_b for
    this core's 64 head rows (head selection via a one-hot matmul).
Matmul operands are float32r (TF32-like: 2x stream rate of fp32, ~1e-4
rounding); PRE = W_ih@x + b is stored bf16 and injected into the gate PSUM
via identity-matmuls and DVE adds (split to balance the two engines).
"""
import sys
sys.path.insert(0, '/opt/trn_rl_repo')
import numpy as np

import concourse.bass as bass
import concourse.mybir as mybir
import concourse.tile as tile
from concourse import bacc
from concourse.masks import make_identity
from concourse.bass_utils import run_bass_kernel_spmd

F32 = mybir.dt.float32
F32R = mybir.dt.float32r
BF16 = mybir.dt.bfloat16
I16 = mybir.dt.int16
AF = mybir.ActivationFunctionType
OP = mybir.AluOpType

L = 512
NG = 2048
V2 = 25000
WD, PD = 256, 64
DIN = WD + PD
M = 512
NCORES = 8
HPC = L // NCORES
K0 = 10
K1 = 10
PAIR_ORDER = (0, 2, 4, 6, 1, 3, 5, 7)   # i,f,g,o for h0/h1 first, then h2/h3
N_TE_INJECT = 2                          # pre-inject via TensorE for last halves

_CACHE = {}


def _emit_lstm_dir(nc, sp_, wp, gp, psum, identr, zsrc, pre, whh, n_sweeps):
    """Jacobi-iterate one LSTM direction; returns [128, 4, 512] f32r hidden."""
    sig, tanh = AF.Sigmoid, AF.Tanh
    hA = sp_.tile([128, 4, L], F32R, tag="hA")
    hB = sp_.tile([128, 4, L], F32R, tag="hB")
    nc.vector.tensor_copy(hA[:, :, 0:1], zsrc[:, 0:1].to_broadcast([128, 4, 1]))
    nc.vector.tensor_copy(hB[:, :, 0:1], zsrc[:, 0:1].to_broadcast([128, 4, 1]))
    G = {}
    for g in range(4):
        G[g] = gp.tile([128, 4, L], F32, tag=f"G{g}", name=f"G{g}")
    for s in range(n_sweeps):
        hprev, hnew = (hA, hB) if s % 2 == 0 else (hB, hA)
        last = s == n_sweeps - 1
        for a in PAIR_ORDER:        # n-tile pair (2a, 2a+1), same gate
            gate = (2 * a) // 4
            func = tanh if gate == 2 else sig
            dst = G[gate][:, (2 * a) % 4:(2 * a) % 4 + 2, :]
            if s == 0:
                nc.scalar.activation(dst, pre[:, 2 * a:2 * a + 2, :], func)
            else:
                for half in range(2):
                    nt = 2 * a + half
                    zph = psum.tile([128, L], F32, tag="zp", name=f"zp{nt}")
                    te_inject = nt >= 16 - N_TE_INJECT
                    for kt in range(4):
                        nc.tensor.matmul(zph[:], whh[:, kt, nt * 128:(nt + 1) * 128],
                                         hprev[:, kt, :], start=(kt == 0),
                                         stop=(kt == 3 and not te_inject))
                    if te_inject:
                        nc.tensor.matmul(zph[:], identr[:], pre[:, nt, :],
                                         start=False, stop=True)
                    else:
                        nc.vector.tensor_tensor(zph[:], zph[:], pre[:, nt, :],
                                                OP.add)
                    nc.scalar.activation(dst[:, half, :], zph[:], func)
        for ht in range(4):
            u = wp.tile([128, L], F32, tag="u")
            nc.vector.tensor_tensor(u[:], G[0][:, ht, :], G[2][:, ht, :], OP.mult)
            c = wp.tile([128, L], F32, tag="c")
            nc.vector.tensor_tensor_scan(c[:], G[1][:, ht, :], u[:], 0.0,
                                         OP.mult, OP.add)
            tc_ = wp.tile([128, L], F32, tag="tc")
            nc.scalar.activation(tc_[:], c[:], tanh)
            if last:
                nc.vector.tensor_tensor(hnew[:, ht, :], G[3][:, ht, :], tc_[:],
                                        OP.mult)
            else:
                nc.vector.tensor_tensor(hnew[:, ht, 1:L], G[3][:, ht, 0:L - 1],
                                        tc_[:, 0:L - 1], OP.mult)
    return hA if n_sweeps % 2 == 0 else hB


def _build_program():
    nc = bacc.Bacc("TRN2", target_bir_lowering=False, debug=False,
                   num_devices=NCORES)

    def dram_in(name, shape, dtype=F32):
        return nc.dram_tensor(name, shape, dtype, kind="ExternalInput")

    w2_d = dram_in("w2", [V2, 2 * WD])
    pemb_d = dram_in("pemb", [50, PD])
    widx_d = dram_in("widx", [128, 32], I16)
    pidx_d = dram_in("pidx", [128, 32], I16)
    wpar_d = dram_in("wpar", [128, 4, 1])
    wih0_d = dram_in("wih0", [128, 3, NG], F32R)
    whh0_d = dram_in("whh0", [128, 4, NG], F32R)
    b0_d = dram_in("b0", [128, 16])
    wih1_d = dram_in("wih1", [128, 8, NG], F32R)
    whh1_d = dram_in("whh1", [128, 4, NG], F32R)
    b1_d = dram_in("b1", [128, 16])
    mlpw_d = dram_in("mlpw", [128, 8, M], F32R)
    mlpb2_d = dram_in("mlpb2", [128, 4])
    outw_d = dram_in("outw", [128, 4, 128], F32R)
    outb_d = dram_in("outb", [1, 1])
    sel_d = dram_in("sel", [128, 4, HPC], F32R)
    mi_d = dram_in("maskI", [128, 1])
    mr_d = dram_in("maskR", [128, 1])
    out_d = nc.dram_tensor("out", [HPC, L], F32, kind="ExternalOutput")

    GROUPS = [[0, 4], [1, 5], [2, 6], [3, 7]]

    with tile.TileContext(nc) as tc:
        with (
            tc.tile_pool(name="pp", bufs=1) as pp,
            tc.tile_pool(name="wp", bufs=2) as wp,
            tc.tile_pool(name="gp", bufs=1) as gp,
            tc.tile_pool(name="ep", bufs=1) as ep,
            tc.tile_pool(name="dram", bufs=2, space="DRAM") as dp,
            tc.tile_pool(name="psA", bufs=5, space="PSUM") as psum,
            tc.tile_pool(name="psB", bufs=1, space="PSUM") as psum2,
            tc.tile_pool(name="psC", bufs=2, space="PSUM") as psum3,
        ):
            ident = pp.tile([128, 128], F32, tag="ident")
            make_identity(nc, ident[:])
            identr = pp.tile([128, 128], BF16, tag="identr")
            nc.vector.tensor_copy(identr[:], ident[:])
            zsrc = pp.tile([128, 1], F32, tag="zsrc")
            nc.vector.memset(zsrc[:], 0.0)
            mI = pp.tile([128, 1], F32, tag="mI")
            mR = pp.tile([128, 1], F32, tag="mR")
            nc.sync.dma_start(mI[:], mi_d[:])
            nc.sync.dma_start(mR[:], mr_d[:])
            x1T = pp.tile([128, 8, L], F32R, tag="x1T")
            x2T = pp.tile([128, 8, L], F32R, tag="x2T")

            def compute_pre(pre, wih, src, bT, nkt):
                for nt in range(16):
                    zph = psum.tile([128, L], F32, tag="zp", name=f"prez{nt}")
                    for kt in range(nkt):
                        nc.tensor.matmul(zph, wih[:, kt, nt * 128:(nt + 1) * 128],
                                         src[:, kt, :], start=(kt == 0),
                                         stop=(kt == nkt - 1))
                    nc.scalar.activation(pre[:, nt, :], zph, AF.Identity,
                                         bias=bT[:, nt:nt + 1])

            def exchange(hfin, xdst):
                """AllGather my hidden with my partner; build [128, 8, L]
                bidirectional input in MY time orientation via 0/1 masks."""
                inb = dp.tile([128, 4, L], F32R, tag="inb")
                outb_t = dp.tile([2, 128, 4, L], F32R, tag="outb_t")
                nc.sync.dma_start(inb[:], hfin[:])
                nc.gpsimd.collective_compute(
                    "AllGather", OP.bypass, replica_groups=GROUPS,
                    ins=[inb.opt()], outs=[outb_t.opt()])
                s0 = ep.tile([128, 4, L], F32R, tag="slot0")
                s1 = ep.tile([128, 4, L], F32R, tag="slot1")
                nc.sync.dma_start(s0[:], outb_t[0])
                nc.sync.dma_start(s1[:], outb_t[1])
                tmp = ep.tile([128, L], F32, tag="xsel")
                for kt in range(4):
                    nc.vector.tensor_tensor(
                        tmp[:], s0[:, kt, :], mI[:].to_broadcast([128, L]), OP.mult)
                    nc.vector.tensor_tensor(
                        xdst[:, kt, 0:L], s0[:, kt, ::-1],
                        mR[:].to_broadcast([128, L]), OP.mult)
                    nc.vector.tensor_tensor(xdst[:, kt, :], xdst[:, kt, :],
                                            tmp[:], OP.add)
                    nc.vector.tensor_tensor(
                        tmp[:], s1[:, kt, ::-1], mI[:].to_broadcast([128, L]),
                        OP.mult)
                    nc.vector.tensor_tensor(
                        xdst[:, 4 + kt, 0:L], s1[:, kt, :],
                        mR[:].to_broadcast([128, L]), OP.mult)
                    nc.vector.tensor_tensor(xdst[:, 4 + kt, :],
                                            xdst[:, 4 + kt, :], tmp[:], OP.add)

            # ---------- gather + transpose + layer 0 ----------
            with tc.tile_pool(name="p0a", bufs=1) as p0a:
              pre0 = p0a.tile([128, 16, L], BF16, tag="pre0")
              bT0 = p0a.tile([128, 16], F32, tag="bT0")
              nc.sync.dma_start(bT0[:], b0_d[:])
              with tc.tile_pool(name="pC", bufs=1) as pC, \
                 tc.tile_pool(name="pCw", bufs=1) as pCw:
                widx = pCw.tile([128, 32], I16, tag="widx")
                pidx = pCw.tile([128, 32], I16, tag="pidx")
                wpar = pCw.tile([128, 4, 1], F32, tag="wpar")
                nc.sync.dma_start(widx[:], widx_d[:])
                nc.sync.dma_start(pidx[:], pidx_d[:])
                nc.sync.dma_start(wpar[:], wpar_d[:])
                pair = pCw.tile([128, 4, 2 * WD], F32, tag="pair")
                nc.gpsimd.dma_gather(pair[:], w2_d[:], widx[:], L, L,
                                     elem_size=2 * WD)
                xp = pCw.tile([128, 4, PD], F32, tag="xp")
                nc.gpsimd.dma_gather(xp[:], pemb_d[:], pidx[:], L, L,
                                     elem_size=PD)
                x = pCw.tile([128, 4, DIN], F32, tag="x")
                xw = x[:, :, 0:WD]
                nc.vector.tensor_tensor(xw, pair[:, :, WD:2 * WD],
                                        pair[:, :, 0:WD], OP.subtract)
                nc.vector.tensor_tensor(xw, xw,
                                        wpar[:].to_broadcast([128, 4, WD]), OP.mult)
                nc.vector.tensor_tensor(xw, xw, pair[:, :, 0:WD], OP.add)
                nc.vector.tensor_copy(x[:, :, WD:DIN], xp[:])
                xT = pC.tile([128, 3, L], F32R, tag="xT")
                nc.vector.tensor_copy(
                    xT[64:128, 2, :], zsrc[64:128, 0:1].to_broadcast([64, L]))
                for ct in range(4):
                    for dblk, wdt in ((0, 128), (1, 128), (2, 64)):
                        tp = psum2.tile([128, 128], F32, tag="ps2")
                        nc.tensor.transpose(
                            tp[0:wdt, :], x[:, ct, dblk * 128:dblk * 128 + wdt],
                            ident[:])
                        nc.vector.tensor_copy(
                            xT[0:wdt, dblk, ct * 128:(ct + 1) * 128], tp[0:wdt, :])

                with tc.tile_pool(name="pw0", bufs=1) as pw0:
                    wih0 = pw0.tile([128, 3, NG], F32R, tag="wih0")
                    nc.sync.dma_start(wih0[:], wih0_d[:])
                    compute_pre(pre0, wih0, xT, bT0, 3)

              with tc.tile_pool(name="p0b", bufs=1) as p0b:
                  whh0 = p0b.tile([128, 4, NG], F32R, tag="whh0")
                  nc.sync.dma_start(whh0[:], whh0_d[:])
                  hfin0 = _emit_lstm_dir(nc, p0b, wp, gp, psum, identr, zsrc,
                                         pre0, whh0, K0)
                  exchange(hfin0, x1T)

            # ---------- layer 1 ----------
            with tc.tile_pool(name="p1a", bufs=1) as p1a:
                pre1 = p1a.tile([128, 16, L], BF16, tag="pre1")
                bT1 = p1a.tile([128, 16], F32, tag="bT1")
                nc.sync.dma_start(bT1[:], b1_d[:])
                with tc.tile_pool(name="p1w", bufs=1) as p1w:
                    for nh in range(2):
                        wih1 = p1w.tile([128, 8, NG // 2], F32R, tag="wih1")
                        nc.sync.dma_start(
                            wih1[:],
                            wih1_d[:, :, nh * (NG // 2):(nh + 1) * (NG // 2)])
                        for nt in range(nh * 8, (nh + 1) * 8):
                            zph = psum.tile([128, L], F32, tag="zp",
                                            name=f"prez{nh}_{nt}")
                            for kt in range(8):
                                nc.tensor.matmul(
                                    zph,
                                    wih1[:, kt, (nt - nh * 8) * 128:(nt - nh * 8 + 1) * 128],
                                    x1T[:, kt, :], start=(kt == 0), stop=(kt == 7))
                            nc.scalar.activation(pre1[:, nt, :], zph, AF.Identity,
                                                 bias=bT1[:, nt:nt + 1])
                with tc.tile_pool(name="p1b", bufs=1) as p1b:
                    whh1 = p1b.tile([128, 4, NG], F32R, tag="whh1")
                    nc.sync.dma_start(whh1[:], whh1_d[:])
                    hfin1 = _emit_lstm_dir(nc, p1b, wp, gp, psum, identr, zsrc,
                                           pre1, whh1, K1)
                    exchange(hfin1, x2T)

            # ---------- MLP + pairwise ----------
            with tc.tile_pool(name="pF", bufs=1) as pF, \
                 tc.tile_pool(name="pFw", bufs=2) as pFw:
                mlpw = pF.tile([128, 8, M], F32R, tag="mlpw")
                nc.sync.dma_start(mlpw[:], mlpw_d[:])
                mlp_mt = pF.tile([128, 4, L], F32R, tag="mlp_mt")
                mlp_tm = pF.tile([128, 4, M], F32R, tag="mlp_tm")
                for mt in range(4):
                    zph = psum.tile([128, L], F32, tag="zp", name=f"mlpz{mt}")
                    for kt in range(8):
                        nc.tensor.matmul(zph, mlpw[:, kt, mt * 128:(mt + 1) * 128],
                                         x2T[:, kt, :], start=(kt == 0),
                                         stop=(kt == 7))
                    nc.scalar.activation(mlp_mt[:, mt, :], zph, AF.Copy)
                for tt in range(4):
                    zph = psum.tile([128, L], F32, tag="zp", name=f"mlptz{tt}")
                    for kt in range(8):
                        nc.tensor.matmul(zph, x2T[:, kt, tt * 128:(tt + 1) * 128],
                                         mlpw[:, kt, :], start=(kt == 0),
                                         stop=(kt == 7))
                    nc.scalar.activation(mlp_tm[:, tt, :], zph, AF.Copy)

                sel = pF.tile([128, 4, HPC], F32R, tag="sel")
                nc.sync.dma_start(sel[:], sel_d[:])
                mlpb2 = pF.tile([128, 4], F32, tag="mlpb2")
                nc.sync.dma_start(mlpb2[:], mlpb2_d[:])
                myb = pF.tile([128, 4, HPC], F32, tag="myb")
                for mt in range(4):
                    zp = psum2.tile([128, HPC], F32, tag="ps2")
                    for tt in range(4):
                        nc.tensor.matmul(zp[:],
                                         mlp_tm[:, tt, mt * 128:(mt + 1) * 128],
                                         sel[:, tt, :], start=(tt == 0),
                                         stop=(tt == 3))
                    nc.scalar.activation(myb[:, mt, :], zp[:], AF.Identity,
                                         bias=mlpb2[:, mt:mt + 1])

                outw = pF.tile([128, 4, 128], F32R, tag="outw")
                nc.sync.dma_start(outw[:], outw_d[:])
                outb = pF.tile([1, 1], F32, tag="outb")
                nc.sync.dma_start(outb[:], outb_d[:])
                for j in range(HPC):
                    sp = psum3.tile([128, L], F32, tag="sp")
                    for mt in range(4):
                        tt_ = pFw.tile([128, L], F32R, tag="T")
                        nc.scalar.activation(tt_[:], mlp_mt[:, mt, :], AF.Tanh,
                                             bias=myb[:, mt, j:j + 1])
                        nc.tensor.matmul(sp[:], outw[:, mt, :], tt_[:],
                                         start=(mt == 0), stop=(mt == 3))
                    srow = pFw.tile([1, L], F32, tag="srow")
                    nc.vector.tensor_tensor(
                        srow[:], sp[0:1, :], outb[:].to_broadcast([1, L]), OP.add)
                    nc.sync.dma_start(out_d[j:j + 1, :], srow[:])

    nc.compile()
    return nc


def _packT(W, ktiles, pad_to=None):
    WT = np.ascontiguousarray(np.asarray(W).T.astype(np.float32))
    k, n = WT.shape
    if pad_to is not None and k < pad_to:
        WT = np.vstack([WT, np.zeros((pad_to - k, n), np.float32)])
    return np.ascontiguousarray(WT.reshape(ktiles, 128, n).transpose(1, 0, 2))


def _wrap16(idx):
    a = np.asarray(idx).astype(np.int64).reshape(32, 16).T.astype(np.int16)
    return np.ascontiguousarray(np.tile(a, (8, 1)))


def _packow(w):
    ow = np.zeros((128, 4, 128), np.float32)
    ow[:, :, 0] = _bpack(w, 4)
    return np.ascontiguousarray(ow)


def _bpack(b, tiles):
    return np.ascontiguousarray(np.asarray(b, np.float32).reshape(tiles, 128).T)


def kernel(**inputs):
    if "nc" not in _CACHE:
        _CACHE["nc"] = _build_program()
    nc = _CACHE["nc"]

    inp = {k: np.asarray(v) for k, v in inputs.items()}
    widx = inp["word_idx"].astype(np.int64)
    pidx = inp["pos_idx"].astype(np.int64)

    base = {
        "w2": np.ascontiguousarray(
            inp["w_embed"].astype(np.float32).reshape(V2, 2 * WD)),
        "pemb": np.ascontiguousarray(inp["p_embed"].astype(np.float32)),
        "mlpw": _packT(inp["mlp_W"], 8),
        "mlpb2": _bpack(2.0 * inp["mlp_b"], 4),
        "outw": _packow(inp["out_w"]),
        "outb": np.asarray(inp["out_b"], np.float32).reshape(1, 1),
    }

    def dir_inputs(rev):
        w = widx[::-1] if rev else widx
        p = pidx[::-1] if rev else pidx
        sfx = "r" if rev else ""
        return {
            "widx": _wrap16(w // 2),
            "pidx": _wrap16(p),
            "wpar": np.ascontiguousarray(
                (w % 2).astype(np.float32).reshape(4, 128).T.reshape(128, 4, 1)),
            "wih0": _packT(inp[f"W_ih_l0{sfx}"], 3, pad_to=384),
            "whh0": _packT(inp[f"W_hh_l0{sfx}"], 4),
            "b0": _bpack(inp[f"b_ih_l0{sfx}"] + inp[f"b_hh_l0{sfx}"], 16),
            "wih1": _packT(inp[f"W_ih_l1{sfx}"], 8),
            "whh1": _packT(inp[f"W_hh_l1{sfx}"], 4),
            "b1": _bpack(inp[f"b_ih_l1{sfx}"] + inp[f"b_hh_l1{sfx}"], 16),
            "maskI": np.full((128, 1), 0.0 if rev else 1.0, np.float32),
            "maskR": np.full((128, 1), 1.0 if rev else 0.0, np.float32),
        }

    fwd_in, bwd_in = dir_inputs(False), dir_inputs(True)

    in_maps = []
    for c in range(NCORES):
        rev = c >= 4
        m = dict(base)
        m.update(bwd_in if rev else fwd_in)
        sel = np.zeros((L, HPC), np.float32)
        for j in range(HPC):
            i = c * HPC + j                        # global head row
            t = (L - 1 - i) if rev else i          # row in this core's time
            sel[t, j] = 1.0
        m["sel"] = np.ascontiguousarray(
            sel.reshape(4, 128, HPC).transpose(1, 0, 2))
        in_maps.append(m)

    res = run_bass_kernel_spmd(nc, in_maps, list(range(NCORES)))
    out = np.empty((L, L - 1), np.float32)
    for c in range(NCORES):
        o = res.results[c]["out"]                  # [HPC, 512] in core time
        for j in range(HPC):
            i = c * HPC + j
            if c >= 4:
                out[i] = o[j, 0:L - 1][::-1]
            else:
                out[i] = o[j, 1:L]
    return out


# revision 15
# speedup vs baseline: 4.2683x; 1.0966x over previous
"""Trainium2 Bass kernel for nn_DependencyParser.

SPMD over 8 NeuronCores; cores 0-3 run the forward LSTM direction, cores 4-7
the backward direction -- one identical program, direction expressed purely
through per-core DATA (time-reversed gather indices, direction-specific
weights, 0/1 orientation masks).  Per core:
  - on-device embedding gather (dma_gather; paired-row trick since the 50000
    vocab exceeds the int16 index range), PE-transpose to [feature, time]
  - 2-layer LSTM via Jacobi fixed-point iteration over the hidden sequence:
    each sweep is a batched [2048,512]x[512,512] matmul + gate activations;
    the cell state is computed EXACTLY per sweep with the hardware linear
    scan op (tensor_tensor_scan).  Contraction ~0.55x/sweep; K=10 sweeps.
  - after each layer the two directions exchange hidden states with a
    pairwise AllGather; each core rebuilds the bidirectional input in its
    own time orientation with 0/1 mask combines (reversal = negative-stride
    DVE copies).
  - head MLP, then pairwise scores tanh(mlp[i]+mlp[j+1]) @ out_w + out_b for
    this core's 64 head rows (head selection via a one-hot matmul).
Matmul operands are float32r (TF32-like: 2x stream rate of fp32, ~1e-4
rounding); PRE = W_ih@x + b is stored bf16 and injected into the gate PSUM
via identity-matmuls and DVE adds (split to balance the two engines).
"""
import sys
sys.path.insert(0, '/opt/trn_rl_repo')
import numpy as np

import concourse.bass as bass
import concourse.mybir as mybir
import concourse.tile as tile
from concourse import bacc
from concourse.masks import make_identity
from concourse.bass_utils import run_bass_kernel_spmd

F32 = mybir.dt.float32
F32R = mybir.dt.float16  # fp16: 1-pass LDWEIGHTS, 2x stream, ~5e-4 rounding
BF16 = mybir.dt.bfloat16
I16 = mybir.dt.int16
AF = mybir.ActivationFunctionType
OP = mybir.AluOpType

L = 512
NG = 2048
V2 = 25000
WD, PD = 256, 64
DIN = WD + PD
M = 512
NCORES = 8
HPC = L // NCORES
K0 = 10
K1 = 10
PAIR_ORDER = (0, 2, 4, 6, 1, 3, 5, 7)   # i,f,g,o for h0/h1 first, then h2/h3
N_TE_INJECT = 2                          # pre-inject via TensorE for last halves

_CACHE = {}


def _emit_lstm_dir(nc, sp_, wp, gp, psum, identr, zsrc, pre, whh, n_sweeps):
    """Jacobi-iterate one LSTM direction; returns [128, 4, 512] f32r hidden."""
    sig, tanh = AF.Sigmoid, AF.Tanh
    hA = sp_.tile([128, 4, L], F32R, tag="hA")
    hB = sp_.tile([128, 4, L], F32R, tag="hB")
    nc.vector.tensor_copy(hA[:, :, 0:1], zsrc[:, 0:1].to_broadcast([128, 4, 1]))
    nc.vector.tensor_copy(hB[:, :, 0:1], zsrc[:, 0:1].to_broadcast([128, 4, 1]))
    G = {}
    for g in range(4):
        G[g] = gp.tile([128, 4, L], F32, tag=f"G{g}", name=f"G{g}")
    for s in range(n_sweeps):
        hprev, hnew = (hA, hB) if s % 2 == 0 else (hB, hA)
        last = s == n_sweeps - 1
        for a in PAIR_ORDER:        # n-tile pair (2a, 2a+1), same gate
            gate = (2 * a) // 4
            func = tanh if gate == 2 else sig
            dst = G[gate][:, (2 * a) % 4:(2 * a) % 4 + 2, :]
            if s == 0:
                nc.scalar.activation(dst, pre[:, 2 * a:2 * a + 2, :], func)
            else:
                for half in range(2):
                    nt = 2 * a + half
                    zph = psum.tile([128, L], F32, tag="zp", name=f"zp{nt}")
                    te_inject = nt >= 16 - N_TE_INJECT
                    for kt in range(4):
                        nc.tensor.matmul(zph[:], whh[:, kt, nt * 128:(nt + 1) * 128],
                                         hprev[:, kt, :], start=(kt == 0),
                                         stop=(kt == 3 and not te_inject))
                    if te_inject:
                        nc.tensor.matmul(zph[:], identr[:], pre[:, nt, :],
                                         start=False, stop=True)
                    else:
                        nc.vector.tensor_tensor(zph[:], zph[:], pre[:, nt, :],
                                                OP.add)
                    nc.scalar.activation(dst[:, half, :], zph[:], func)
        for ht in range(4):
            u = wp.tile([128, L], F32, tag="u")
            nc.vector.tensor_tensor(u[:], G[0][:, ht, :], G[2][:, ht, :], OP.mult)
            c = wp.tile([128, L], F32, tag="c")
            nc.vector.tensor_tensor_scan(c[:], G[1][:, ht, :], u[:], 0.0,
                                         OP.mult, OP.add)
            tc_ = wp.tile([128, L], F32, tag="tc")
            nc.scalar.activation(tc_[:], c[:], tanh)
            if last:
                nc.vector.tensor_tensor(hnew[:, ht, :], G[3][:, ht, :], tc_[:],
                                        OP.mult)
            else:
                nc.vector.tensor_tensor(hnew[:, ht, 1:L], G[3][:, ht, 0:L - 1],
                                        tc_[:, 0:L - 1], OP.mult)
    return hA if n_sweeps % 2 == 0 else hB


def _build_program():
    nc = bacc.Bacc("TRN2", target_bir_lowering=False, debug=False,
                   num_devices=NCORES)

    def dram_in(name, shape, dtype=F32):
        return nc.dram_tensor(name, shape, dtype, kind="ExternalInput")

    w2_d = dram_in("w2", [V2, 2 * WD])
    pemb_d = dram_in("pemb", [50, PD])
    widx_d = dram_in("widx", [128, 32], I16)
    pidx_d = dram_in("pidx", [128, 32], I16)
    wpar_d = dram_in("wpar", [128, 4, 1])
    wih0_d = dram_in("wih0", [128, 3, NG], F32R)
    whh0_d = dram_in("whh0", [128, 4, NG], F32R)
    b0_d = dram_in("b0", [128, 16])
    wih1_d = dram_in("wih1", [128, 8, NG], F32R)
    whh1_d = dram_in("whh1", [128, 4, NG], F32R)
    b1_d = dram_in("b1", [128, 16])
    mlpw_d = dram_in("mlpw", [128, 8, M], F32R)
    mlpb2_d = dram_in("mlpb2", [128, 4])
    outw_d = dram_in("outw", [128, 4, 128], F32R)
    outb_d = dram_in("outb", [1, 1])
    sel_d = dram_in("sel", [128, 4, HPC], F32R)
    mi_d = dram_in("maskI", [128, 1])
    mr_d = dram_in("maskR", [128, 1])
    out_d = nc.dram_tensor("out", [HPC, L], F32, kind="ExternalOutput")

    GROUPS = [[0, 4], [1, 5], [2, 6], [3, 7]]

    with tile.TileContext(nc) as tc:
        with (
            tc.tile_pool(name="pp", bufs=1) as pp,
            tc.tile_pool(name="wp", bufs=2) as wp,
            tc.tile_pool(name="gp", bufs=1) as gp,
            tc.tile_pool(name="ep", bufs=1) as ep,
            tc.tile_pool(name="dram", bufs=2, space="DRAM") as dp,
            tc.tile_pool(name="psA", bufs=5, space="PSUM") as psum,
            tc.tile_pool(name="psB", bufs=1, space="PSUM") as psum2,
            tc.tile_pool(name="psC", bufs=2, space="PSUM") as psum3,
        ):
            ident = pp.tile([128, 128], F32, tag="ident")
            make_identity(nc, ident[:])
            identr = pp.tile([128, 128], BF16, tag="identr")
            nc.vector.tensor_copy(identr[:], ident[:])
            zsrc = pp.tile([128, 1], F32, tag="zsrc")
            nc.vector.memset(zsrc[:], 0.0)
            mI = pp.tile([128, 1], F32, tag="mI")
            mR = pp.tile([128, 1], F32, tag="mR")
            nc.sync.dma_start(mI[:], mi_d[:])
            nc.sync.dma_start(mR[:], mr_d[:])
            x1T = pp.tile([128, 8, L], F32R, tag="x1T")
            x2T = pp.tile([128, 8, L], F32R, tag="x2T")

            def compute_pre(pre, wih, src, bT, nkt):
                for nt in range(16):
                    zph = psum.tile([128, L], F32, tag="zp", name=f"prez{nt}")
                    for kt in range(nkt):
                        nc.tensor.matmul(zph, wih[:, kt, nt * 128:(nt + 1) * 128],
                                         src[:, kt, :], start=(kt == 0),
                                         stop=(kt == nkt - 1))
                    nc.scalar.activation(pre[:, nt, :], zph, AF.Identity,
                                         bias=bT[:, nt:nt + 1])

            def exchange(hfin, xdst):
                """AllGather my hidden with my partner; build [128, 8, L]
                bidirectional input in MY time orientation via 0/1 masks."""
                inb = dp.tile([128, 4, L], F32R, tag="inb")
                outb_t = dp.tile([2, 128, 4, L], F32R, tag="outb_t")
                nc.sync.dma_start(inb[:], hfin[:])
                nc.gpsimd.collective_compute(
                    "AllGather", OP.bypass, replica_groups=GROUPS,
                    ins=[inb.opt()], outs=[outb_t.opt()])
                s0 = ep.tile([128, 4, L], F32R, tag="slot0")
                s1 = ep.tile([128, 4, L], F32R, tag="slot1")
                nc.sync.dma_start(s0[:], outb_t[0])
                nc.sync.dma_start(s1[:], outb_t[1])
                tmp = ep.tile([128, L], F32, tag="xsel")
                for kt in range(4):
                    nc.vector.tensor_tensor(
                        tmp[:], s0[:, kt, :], mI[:].to_broadcast([128, L]), OP.mult)
                    nc.vector.tensor_tensor(
                        xdst[:, kt, 0:L], s0[:, kt, ::-1],
                        mR[:].to_broadcast([128, L]), OP.mult)
                    nc.vector.tensor_tensor(xdst[:, kt, :], xdst[:, kt, :],
                                            tmp[:], OP.add)
                    nc.vector.tensor_tensor(
                        tmp[:], s1[:, kt, ::-1], mI[:].to_broadcast([128, L]),
                        OP.mult)
                    nc.vector.tensor_tensor(
                        xdst[:, 4 + kt, 0:L], s1[:, kt, :],
                        mR[:].to_broadcast([128, L]), OP.mult)
                    nc.vector.tensor_tensor(xdst[:, 4 + kt, :],
                                            xdst[:, 4 + kt, :], tmp[:], OP.add)

            # ---------- gather + transpose + layer 0 ----------
            with tc.tile_pool(name="p0a", bufs=1) as p0a:
              pre0 = p0a.tile([128, 16, L], BF16, tag="pre0")
              bT0 = p0a.tile([128, 16], F32, tag="bT0")
              nc.sync.dma_start(bT0[:], b0_d[:])
              with tc.tile_pool(name="pC", bufs=1) as pC, \
                 tc.tile_pool(name="pCw", bufs=1) as pCw:
                widx = pCw.tile([128, 32], I16, tag="widx")
                pidx = pCw.tile([128, 32], I16, tag="pidx")
                wpar = pCw.tile([128, 4, 1], F32, tag="wpar")
                nc.sync.dma_start(widx[:], widx_d[:])
                nc.sync.dma_start(pidx[:], pidx_d[:])
                nc.sync.dma_start(wpar[:], wpar_d[:])
                pair = pCw.tile([128, 4, 2 * WD], F32, tag="pair")
                nc.gpsimd.dma_gather(pair[:], w2_d[:], widx[:], L, L,
                                     elem_size=2 * WD)
                xp = pCw.tile([128, 4, PD], F32, tag="xp")
                nc.gpsimd.dma_gather(xp[:], pemb_d[:], pidx[:], L, L,
                                     elem_size=PD)
                x = pCw.tile([128, 4, DIN], F32, tag="x")
                xw = x[:, :, 0:WD]
                nc.vector.tensor_tensor(xw, pair[:, :, WD:2 * WD],
                                        pair[:, :, 0:WD], OP.subtract)
                nc.vector.tensor_tensor(xw, xw,
                                        wpar[:].to_broadcast([128, 4, WD]), OP.mult)
                nc.vector.tensor_tensor(xw, xw, pair[:, :, 0:WD], OP.add)
                nc.vector.tensor_copy(x[:, :, WD:DIN], xp[:])
                xT = pC.tile([128, 3, L], F32R, tag="xT")
                nc.vector.tensor_copy(
                    xT[64:128, 2, :], zsrc[64:128, 0:1].to_broadcast([64, L]))
                for ct in range(4):
                    for dblk, wdt in ((0, 128), (1, 128), (2, 64)):
                        tp = psum2.tile([128, 128], F32, tag="ps2")
                        nc.tensor.transpose(
                            tp[0:wdt, :], x[:, ct, dblk * 128:dblk * 128 + wdt],
                            ident[:])
                        nc.vector.tensor_copy(
                            xT[0:wdt, dblk, ct * 128:(ct + 1) * 128], tp[0:wdt, :])

                with tc.tile_pool(name="pw0", bufs=1) as pw0:
                    wih0 = pw0.tile([128, 3, NG], F32R, tag="wih0")
                    nc.sync.dma_start(wih0[:], wih0_d[:])
                    compute_pre(pre0, wih0, xT, bT0, 3)

              with tc.tile_pool(name="p0b", bufs=1) as p0b:
                  whh0 = p0b.tile([128, 4, NG], F32R, tag="whh0")
                  nc.sync.dma_start(whh0[:], whh0_d[:])
                  hfin0 = _emit_lstm_dir(nc, p0b, wp, gp, psum, identr, zsrc,
                                         pre0, whh0, K0)
                  exchange(hfin0, x1T)

            # ---------- layer 1 ----------
            with tc.tile_pool(name="p1a", bufs=1) as p1a:
                pre1 = p1a.tile([128, 16, L], BF16, tag="pre1")
                bT1 = p1a.tile([128, 16], F32, tag="bT1")
                nc.sync.dma_start(bT1[:], b1_d[:])
                with tc.tile_pool(name="p1w", bufs=1) as p1w:
                    for nh in range(2):
                        wih1 = p1w.tile([128, 8, NG // 2], F32R, tag="wih1")
                        nc.sync.dma_start(
                            wih1[:],
                            wih1_d[:, :, nh * (NG // 2):(nh + 1) * (NG // 2)])
                        for nt in range(nh * 8, (nh + 1) * 8):
                            zph = psum.tile([128, L], F32, tag="zp",
                                            name=f"prez{nh}_{nt}")
                            for kt in range(8):
                                nc.tensor.matmul(
                                    zph,
                                    wih1[:, kt, (nt - nh * 8) * 128:(nt - nh * 8 + 1) * 128],
                                    x1T[:, kt, :], start=(kt == 0), stop=(kt == 7))
                            nc.scalar.activation(pre1[:, nt, :], zph, AF.Identity,
                                                 bias=bT1[:, nt:nt + 1])
                with tc.tile_pool(name="p1b", bufs=1) as p1b:
                    whh1 = p1b.tile([128, 4, NG], F32R, tag="whh1")
                    nc.sync.dma_start(whh1[:], whh1_d[:])
                    hfin1 = _emit_lstm_dir(nc, p1b, wp, gp, psum, identr, zsrc,
                                           pre1, whh1, K1)
                    exchange(hfin1, x2T)

            # ---------- MLP + pairwise ----------
            with tc.tile_pool(name="pF", bufs=1) as pF, \
                 tc.tile_pool(name="pFw", bufs=2) as pFw:
                mlpw = pF.tile([128, 8, M], F32R, tag="mlpw")
                nc.sync.dma_start(mlpw[:], mlpw_d[:])
                mlp_mt = pF.tile([128, 4, L], F32R, tag="mlp_mt")
                mlp_tm = pF.tile([128, 4, M], F32R, tag="mlp_tm")
                for mt in range(4):
                    zph = psum.tile([128, L], F32, tag="zp", name=f"mlpz{mt}")
                    for kt in range(8):
                        nc.tensor.matmul(zph, mlpw[:, kt, mt * 128:(mt + 1) * 128],
                                         x2T[:, kt, :], start=(kt == 0),
                                         stop=(kt == 7))
                    nc.scalar.activation(mlp_mt[:, mt, :], zph, AF.Copy)
                for tt in range(4):
                    zph = psum.tile([128, L], F32, tag="zp", name=f"mlptz{tt}")
                    for kt in range(8):
                        nc.tensor.matmul(zph, x2T[:, kt, tt * 128:(tt + 1) * 128],
                                         mlpw[:, kt, :], start=(kt == 0),
                                         stop=(kt == 7))
                    nc.scalar.activation(mlp_tm[:, tt, :], zph, AF.Copy)

                sel = pF.tile([128, 4, HPC], F32R, tag="sel")
                nc.sync.dma_start(sel[:], sel_d[:])
                mlpb2 = pF.tile([128, 4], F32, tag="mlpb2")
                nc.sync.dma_start(mlpb2[:], mlpb2_d[:])
                myb = pF.tile([128, 4, HPC], F32, tag="myb")
                for mt in range(4):
                    zp = psum2.tile([128, HPC], F32, tag="ps2")
                    for tt in range(4):
                        nc.tensor.matmul(zp[:],
                                         mlp_tm[:, tt, mt * 128:(mt + 1) * 128],
                                         sel[:, tt, :], start=(tt == 0),
                                         stop=(tt == 3))
                    nc.scalar.activation(myb[:, mt, :], zp[:], AF.Identity,
                                         bias=mlpb2[:, mt:mt + 1])

                outw = pF.tile([128, 4, 128], F32R, tag="outw")
                nc.sync.dma_start(outw[:], outw_d[:])
                outb = pF.tile([1, 1], F32, tag="outb")
                nc.sync.dma_start(outb[:], outb_d[:])
                for j in range(HPC):
                    sp = psum3.tile([128, L], F32, tag="sp")
                    for mt in range(4):
                        tt_ = pFw.tile([128, L], F32R, tag="T")
                        nc.scalar.activation(tt_[:], mlp_mt[:, mt, :], AF.Tanh,
                                             bias=myb[:, mt, j:j + 1])
                        nc.tensor.matmul(sp[:], outw[:, mt, :], tt_[:],
                                         start=(mt == 0), stop=(mt == 3))
                    srow = pFw.tile([1, L], F32, tag="srow")
                    nc.vector.tensor_tensor(
                        srow[:], sp[0:1, :], outb[:].to_broadcast([1, L]), OP.add)
                    nc.sync.dma_start(out_d[j:j + 1, :], srow[:])

    nc.compile()
    return nc


def _packT(W, ktiles, pad_to=None):
    WT = np.ascontiguousarray(np.asarray(W).T.astype(np.float32))
    k, n = WT.shape
    if pad_to is not None and k < pad_to:
        WT = np.vstack([WT, np.zeros((pad_to - k, n), np.float32)])
    return np.ascontiguousarray(
        WT.reshape(ktiles, 128, n).transpose(1, 0, 2).astype(np.float16))


def _wrap16(idx):
    a = np.asarray(idx).astype(np.int64).reshape(32, 16).T.astype(np.int16)
    return np.ascontiguousarray(np.tile(a, (8, 1)))


def _packow(w):
    ow = np.zeros((128, 4, 128), np.float16)
    ow[:, :, 0] = _bpack(w, 4)
    return np.ascontiguousarray(ow)


def _bpack(b, tiles):
    return np.ascontiguousarray(np.asarray(b, np.float32).reshape(tiles, 128).T)


def kernel(**inputs):
    if "nc" not in _CACHE:
        _CACHE["nc"] = _build_program()
    nc = _CACHE["nc"]

    inp = {k: np.asarray(v) for k, v in inputs.items()}
    widx = inp["word_idx"].astype(np.int64)
    pidx = inp["pos_idx"].astype(np.int64)

    base = {
        "w2": np.ascontiguousarray(
            inp["w_embed"].astype(np.float32).reshape(V2, 2 * WD)),
        "pemb": np.ascontiguousarray(inp["p_embed"].astype(np.float32)),
        "mlpw": _packT(inp["mlp_W"], 8),
        "mlpb2": _bpack(2.0 * inp["mlp_b"], 4),
        "outw": _packow(inp["out_w"]),
        "outb": np.asarray(inp["out_b"], np.float32).reshape(1, 1),
    }

    def dir_inputs(rev):
        w = widx[::-1] if rev else widx
        p = pidx[::-1] if rev else pidx
        sfx = "r" if rev else ""
        return {
            "widx": _wrap16(w // 2),
            "pidx": _wrap16(p),
            "wpar": np.ascontiguousarray(
                (w % 2).astype(np.float32).reshape(4, 128).T.reshape(128, 4, 1)),
            "wih0": _packT(inp[f"W_ih_l0{sfx}"], 3, pad_to=384),
            "whh0": _packT(inp[f"W_hh_l0{sfx}"], 4),
            "b0": _bpack(inp[f"b_ih_l0{sfx}"] + inp[f"b_hh_l0{sfx}"], 16),
            "wih1": _packT(inp[f"W_ih_l1{sfx}"], 8),
            "whh1": _packT(inp[f"W_hh_l1{sfx}"], 4),
            "b1": _bpack(inp[f"b_ih_l1{sfx}"] + inp[f"b_hh_l1{sfx}"], 16),
            "maskI": np.full((128, 1), 0.0 if rev else 1.0, np.float32),
            "maskR": np.full((128, 1), 1.0 if rev else 0.0, np.float32),
        }

    fwd_in, bwd_in = dir_inputs(False), dir_inputs(True)

    in_maps = []
    for c in range(NCORES):
        rev = c >= 4
        m = dict(base)
        m.update(bwd_in if rev else fwd_in)
        sel = np.zeros((L, HPC), np.float32)
        for j in range(HPC):
            i = c * HPC + j                        # global head row
            t = (L - 1 - i) if rev else i          # row in this core's time
            sel[t, j] = 1.0
        m["sel"] = np.ascontiguousarray(
            sel.reshape(4, 128, HPC).transpose(1, 0, 2).astype(np.float16))
        in_maps.append(m)

    res = run_bass_kernel_spmd(nc, in_maps, list(range(NCORES)))
    out = np.empty((L, L - 1), np.float32)
    for c in range(NCORES):
        o = res.results[c]["out"]                  # [HPC, 512] in core time
        for j in range(HPC):
            i = c * HPC + j
            if c >= 4:
                out[i] = o[j, 0:L - 1][::-1]
            else:
                out[i] = o[j, 1:L]
    return out
